# revision 20
# baseline (speedup 1.0000x reference)
"""BERT encoder layer on 8 TRN2 NeuronCores (Bass/Tile), data-parallel over batch.

Full inputs: hidden_states [16, 512, 1024], attention_mask [16, 512], weights.
Each core processes 2 batch items (1024 tokens). Weights are replicated; no
collectives. All matmuls run in float32r (~1e-4 rel err vs the fp32 reference).

Layout strategy: activations flow feature-major ("T" suffix = [feature, token])
through the attention block, so stored [in,out] weight matrices are directly
the matmul stationary operand (out = lhsT.T @ rhs) and no weight transposes are
needed. Scores are computed transposed [kt, qt] so the attention mask is a
per-partition bias on the Exp activation, and softmax normalization is deferred:
v is augmented with a ones column so the ctx matmul simultaneously produces the
softmax denominator. LayerNorms run token-major (bn_stats over the free dim);
the two layout flips (input x, and attn_out before the FFN) use PE transposes.

Perf-critical detail (measured on HW): an fp32r matmul whose stationary operand
(lhsT) changes every instruction runs at ~2-5x the ideal 512-cycle spacing,
while back-to-back matmuls REUSING the same lhsT run at ~1.05x ideal. All dense
projections therefore interleave two PSUM accumulation groups so each lhsT tile
is consumed by two consecutive matmuls (the two 512-token/feature halves).
"""

import contextlib

import numpy as np

import concourse.bass as bass
import concourse.mybir as mybir
import concourse.tile as tile
from concourse import bacc
from concourse.bass_utils import run_bass_kernel_spmd
from concourse.masks import make_identity

F32 = mybir.dt.float32
F32R = mybir.dt.float32r
AF = mybir.ActivationFunctionType
OP = mybir.AluOpType

B, S, D, H, F = 16, 512, 1024, 16, 4096
DH = D // H                      # 64
LN_EPS = 1e-12
NCORES = 8
BPC = B // NCORES                # 2 batch items per core
T = BPC * S                      # 1024 tokens per core
P = 128
DSUB = D // P                    # 8
TCH = T // P                     # 8 token chunks
SCH = S // P                     # 4 key chunks per batch item
NT = 512                         # matmul moving-dim tile
NTH = T // NT                    # 2 token halves


class _Pool:
    """Manually-scoped tile pool (pools must close in LIFO stack order)."""

    def __init__(self, tc, name, bufs, space="SBUF"):
        self._cm = tc.tile_pool(name=name, bufs=bufs, space=space)
        self.pool = self._cm.__enter__()

    def tile(self, *a, **k):
        if "name" not in k:
            k["name"] = k.get("tag", "t")
        return self.pool.tile(*a, **k)

    def close(self):
        self._cm.__exit__(None, None, None)


def _load_bias_cols(nc, pool, dram_vec, n_sub, tag, scale=None):
    """[n_sub*P] DRAM vector -> [P, n_sub] SBUF (feature d -> [d%P, d//P])."""
    col = pool.tile([P, n_sub], F32, tag=tag)
    nc.sync.dma_start(col[:], dram_vec.rearrange("(c p) -> p c", p=P))
    if scale is not None:
        nc.vector.tensor_scalar_mul(col[:], col[:], scale)
    return col


def _load_bcast(nc, pool, dram_vec, tag):
    """[D] DRAM vector -> [P, D] SBUF, replicated across partitions."""
    t = pool.tile([P, dram_vec.shape[0]], F32, tag=tag)
    src = bass.AP(tensor=dram_vec.tensor, offset=dram_vec.offset,
                  ap=[[0, P]] + list(dram_vec.ap))
    nc.sync.dma_start(out=t[:], in_=src)
    return t


def _layer_norm_rows(nc, pool, row, g_b, b_b, eps_col, tag):
    """In-place LayerNorm of [P, D] token rows over the free (feature) dim."""
    st = pool.tile([P, 2, 6], F32, tag=f"{tag}_st")
    nc.vector.bn_stats(st[:, 0, :], row[:, 0:512])
    nc.vector.bn_stats(st[:, 1, :], row[:, 512:1024])
    mv = pool.tile([P, 2], F32, tag=f"{tag}_mv")
    nc.vector.bn_aggr(mv[:], st[:])
    std = pool.tile([P, 1], F32, tag=f"{tag}_std")
    nc.scalar.activation(std[:], mv[:, 1:2], AF.Sqrt, bias=eps_col[:], scale=1.0)
    nc.vector.reciprocal(std[:], std[:])
    nc.vector.tensor_scalar(row, row, mv[:, 0:1], std[:], OP.subtract, OP.mult)
    nc.vector.tensor_tensor(row, row, g_b[:], OP.mult)
    nc.vector.tensor_tensor(row, row, b_b[:], OP.add)


def build_bert_layer(tc, loop_n=None):
    nc = tc.nc
    dt = nc.dram_tensor
    x_d = dt("x", [T, D], F32, kind="ExternalInput")
    mask_d = dt("mask", [BPC, S], F32, kind="ExternalInput")
    wq_d = dt("Wq", [D, D], F32, kind="ExternalInput")
    bq_d = dt("bq", [D], F32, kind="ExternalInput")
    wk_d = dt("Wk", [D, D], F32, kind="ExternalInput")
    bk_d = dt("bk", [D], F32, kind="ExternalInput")
    wv_d = dt("Wv", [D, D], F32, kind="ExternalInput")
    bv_d = dt("bv", [D], F32, kind="ExternalInput")
    wo_d = dt("Wo", [D, D], F32, kind="ExternalInput")
    bo_d = dt("bo", [D], F32, kind="ExternalInput")
    g1_d = dt("ln1_g", [D], F32, kind="ExternalInput")
    b1_d = dt("ln1_b", [D], F32, kind="ExternalInput")
    wi_d = dt("Wi", [D, F], F32, kind="ExternalInput")
    bi_d = dt("bi", [F], F32, kind="ExternalInput")
    wo2_d = dt("Wo2", [F, D], F32, kind="ExternalInput")
    bo2_d = dt("bo2", [D], F32, kind="ExternalInput")
    g2_d = dt("ln2_g", [D], F32, kind="ExternalInput")
    b2_d = dt("ln2_b", [D], F32, kind="ExternalInput")
    y_d = dt("y", [T, D], F32, kind="ExternalOutput")

    const = _Pool(tc, "const", 1)
    ident = const.tile([P, P], F32, tag="ident")
    make_identity(nc, ident)
    ident_r = const.tile([P, P], F32R, tag="ident_r")
    nc.vector.tensor_copy(ident_r[:], ident[:])
    zeros_f = const.tile([P, S], F32, tag="zeros_f")
    nc.vector.memset(zeros_f[:], 0.0)
    zeros_r = const.tile([P, S], F32R, tag="zeros_r")
    nc.vector.tensor_copy(zeros_r[:], zeros_f[:])
    ones_col = const.tile([P, 1], F32, tag="ones_col")
    nc.vector.memset(ones_col[:], 1.0)
    ones_r = const.tile([P, 1], F32R, tag="ones_r")
    nc.vector.tensor_copy(ones_r[:], ones_col[:])
    eps_col = const.tile([P, 1], F32, tag="eps")
    nc.vector.memset(eps_col, LN_EPS)
    # per-feature bias columns for feature-major stages (bias = per-partition)
    bqs_col = _load_bias_cols(nc, const, bq_d.ap(), DSUB, "bqs", scale=1.0 / np.sqrt(DH))
    bk_col = _load_bias_cols(nc, const, bk_d.ap(), DSUB, "bk")
    bi_col = _load_bias_cols(nc, const, bi_d.ap(), F // P, "bi")
    # per-feature vectors broadcast across partitions for token-major stages
    bv_b = _load_bcast(nc, const, bv_d.ap(), "bv_b")
    bo_b = _load_bcast(nc, const, bo_d.ap(), "bo_b")
    g1_b = _load_bcast(nc, const, g1_d.ap(), "g1_b")
    b1_b = _load_bcast(nc, const, b1_d.ap(), "b1_b")
    bo2_b = _load_bcast(nc, const, bo2_d.ap(), "bo2_b")
    g2_b = _load_bcast(nc, const, g2_d.ap(), "g2_b")
    b2_b = _load_bcast(nc, const, b2_d.ap(), "b2_b")
    # mask[b, kt] -> [kt%P, b, kt//P] so exp can take it as per-partition bias
    mask_sb = const.tile([P, BPC, SCH], F32, tag="mask")
    for b in range(BPC):
        nc.sync.dma_start(mask_sb[:, b, :],
                          mask_d.ap()[b].rearrange("(c p) -> p c", p=P))

    loop_cm = tc.For_i(0, loop_n, 1) if loop_n else contextlib.nullcontext()
    with loop_cm:
        # fm pool: one [P, DSUB, T] f32r slot shared by Xt (ph0-1), ctxT (ph2-3)
        p_fm = _Pool(tc, "fm", 1)

        # ---- Phase 0: load x token-major, PE-transpose to feature-major Xt ----
        xt = p_fm.tile([P, DSUB, T], F32R, tag="fm")  # Xt[p, ds, t] = x[t, ds*P+p]
        ph0 = _Pool(tc, "ph0", 3)
        ps0 = _Pool(tc, "ps0", 2, space="PSUM")
        for tc_i in range(TCH):
            xtok = ph0.tile([P, D], F32R, tag="xtok")
            nc.gpsimd.dma_start(xtok[:], x_d.ap()[tc_i * P:(tc_i + 1) * P, :])
            for ds in range(DSUB):
                pst = ps0.tile([P, P], F32R, tag="pst")
                nc.tensor.transpose(pst[:], xtok[:, ds * P:(ds + 1) * P], ident_r[:])
                nc.vector.tensor_copy(xt[:, ds, tc_i * P:(tc_i + 1) * P], pst[:])
        ps0.close()
        ph0.close()

        # ---- Phase 1: QKV projections (each lhsT reused by 2 matmuls) ----
        p_qkv = _Pool(tc, "qkv", 1)
        qT = p_qkv.tile([P, DSUB, T], F32R, tag="qT")
        kT = p_qkv.tile([P, DSUB, T], F32R, tag="kT")
        v_sb = p_qkv.tile([P, TCH, D], F32R, tag="v_sb")
        ph1 = _Pool(tc, "ph1", 2)
        ph1w = _Pool(tc, "ph1w", 3)
        ps1 = _Pool(tc, "ps1", 2, space="PSUM")

        for name, w_dram, dst, bias_col, scale in (
            ("q", wq_d, qT, bqs_col, 1.0 / np.sqrt(DH)),
            ("k", wk_d, kT, bk_col, 1.0),
        ):
            wr = w_dram.ap().rearrange("(ks p) m -> p ks m", p=P)
            for mo in range(DSUB):  # output-feature chunk
                wt = ph1w.tile([P, DSUB, P], F32R, tag="w_qkv")
                nc.gpsimd.dma_start(wt[:], wr[:, :, mo * P:(mo + 1) * P])
                psa = ps1.tile([P, NT], F32, tag="ps_qA")
                psb = ps1.tile([P, NT], F32, tag="ps_qB")
                for ks in range(DSUB):
                    nc.tensor.matmul(psa[:], wt[:, ks, :], xt[:, ks, 0:NT],
                                     start=(ks == 0), stop=(ks == DSUB - 1))
                    nc.tensor.matmul(psb[:], wt[:, ks, :], xt[:, ks, NT:T],
                                     start=(ks == 0), stop=(ks == DSUB - 1))
                # dst = scale*psum + scale*bias  (scale folded into bias col)
                nc.scalar.activation(dst[:, mo, 0:NT], psa[:], AF.Identity,
                                     bias=bias_col[:, mo:mo + 1], scale=scale)
                nc.scalar.activation(dst[:, mo, NT:T], psb[:], AF.Identity,
                                     bias=bias_col[:, mo:mo + 1], scale=scale)

        # v token-major: lhsT = Xt token chunk (reused for both Wv halves)
        wvr = wv_d.ap().rearrange("(ks p) m -> p ks m", p=P)
        wv_t = []
        for jh in range(NTH):
            wvt = ph1.tile([P, DSUB, NT], F32R, tag="w_v")
            nc.gpsimd.dma_start(wvt[:], wvr[:, :, jh * NT:(jh + 1) * NT])
            wv_t.append(wvt)
        for tc_i in range(TCH):
            psa = ps1.tile([P, NT], F32, tag="ps_vA")
            psb = ps1.tile([P, NT], F32, tag="ps_vB")
            for ks in range(DSUB):
                lhs = xt[:, ks, tc_i * P:(tc_i + 1) * P]
                nc.tensor.matmul(psa[:], lhs, wv_t[0][:, ks, :],
                                 start=(ks == 0), stop=(ks == DSUB - 1))
                nc.tensor.matmul(psb[:], lhs, wv_t[1][:, ks, :],
                                 start=(ks == 0), stop=(ks == DSUB - 1))
            for jh, ps in ((0, psa), (1, psb)):
                nc.vector.tensor_tensor(
                    v_sb[:, tc_i, jh * NT:(jh + 1) * NT], ps[:],
                    bv_b[:, jh * NT:(jh + 1) * NT], OP.add)
        ps1.close()
        ph1w.close()
        ph1.close()

        # ---- Phase 2: attention ----
        # Scores use K=128: lhsT is a full kT subtile (two heads stacked) and
        # the rhs is q zero-padded outside its head rows, so each kT chunk is a
        # stationary operand REUSED by both heads of the pair (fp32r matmuls
        # with a fresh lhsT each are ~5x slower than reusing ones). The ctx
        # matmuls reuse a [v_even | v_odd] pair block the same way; garbage
        # rows are simply not read back. Softmax denominators come from cheap
        # 2-column one-hot matmuls accumulated per pair.
        ctxT = p_fm.tile([P, DSUB, T], F32R, tag="fm")  # reuses the Xt slot
        ph2 = _Pool(tc, "ph2", 2)
        ps_s = _Pool(tc, "ps_s", 2, space="PSUM")
        ps_ca = _Pool(tc, "ps_ca", 2, space="PSUM")
        ps_cb = _Pool(tc, "ps_cb", 2, space="PSUM")
        ps_d0 = _Pool(tc, "ps_d0", 1, space="PSUM")
        ps_d1 = _Pool(tc, "ps_d1", 1, space="PSUM")
        for b in range(BPC):
            bs = b * S
            for hs in range(DSUB):
                # zero-padded q tiles for the even/odd head of this pair
                pad0 = ph2.tile([P, S], F32R, tag="pad0", bufs=3)
                nc.scalar.copy(pad0[DH:P, :], zeros_r[DH:P, :])
                nc.vector.tensor_copy(pad0[0:DH, :], qT[0:DH, hs, bs:bs + S])
                pad1 = ph2.tile([P, S], F32R, tag="pad1", bufs=3)
                nc.scalar.copy(pad1[0:DH, :], zeros_r[0:DH, :])
                nc.vector.tensor_copy(pad1[DH:P, :], qT[DH:P, hs, bs:bs + S])
                exps = {}
                for c in range(SCH):
                    lhs = kT[:, hs, bs + c * P:bs + (c + 1) * P]
                    for par, pad in ((0, pad0), (1, pad1)):
                        t = ps_s.tile([P, S], F32, tag="pss")
                        nc.tensor.matmul(t[:], lhs, pad[:], start=True, stop=True)
                        e = ph2.tile([P, S], F32R, tag="expT", bufs=8)
                        nc.scalar.activation(e[:], t[:], AF.Exp,
                                             bias=mask_sb[:, b, c:c + 1], scale=1.0)
                        exps[par, c] = e
                psa = ps_ca.tile([P, S], F32, tag="ps_cA")
                psb = ps_cb.tile([P, S], F32, tag="ps_cB")
                for c in range(SCH):
                    vp = v_sb[:, b * SCH + c, hs * P:(hs + 1) * P]
                    nc.tensor.matmul(psa[:], vp, exps[0, c][:],
                                     start=(c == 0), stop=(c == SCH - 1))
                    nc.tensor.matmul(psb[:], vp, exps[1, c][:],
                                     start=(c == 0), stop=(c == SCH - 1))
                psd0 = ps_d0.tile([1, S], F32, tag="ps_den0")
                psd1 = ps_d1.tile([1, S], F32, tag="ps_den1")
                for c in range(SCH):
                    nc.tensor.matmul(psd0[:], ones_r[:], exps[0, c][:],
                                     start=(c == 0), stop=(c == SCH - 1))
                    nc.tensor.matmul(psd1[:], ones_r[:], exps[1, c][:],
                                     start=(c == 0), stop=(c == SCH - 1))
                den0 = ph2.tile([1, S], F32, tag="den0")
                nc.scalar.copy(den0[:], psd0[:])
                den1 = ph2.tile([1, S], F32, tag="den1")
                nc.scalar.copy(den1[:], psd1[:])
                rec0 = ph2.tile([P, S], F32, tag="rec0")
                nc.gpsimd.partition_broadcast(rec0[:], den0[:])
                rec1 = ph2.tile([P, S], F32, tag="rec1")
                nc.gpsimd.partition_broadcast(rec1[:], den1[:])
                nc.vector.reciprocal(rec0[0:DH, :], rec0[0:DH, :])
                nc.vector.reciprocal(rec1[DH:P, :], rec1[DH:P, :])
                nc.vector.tensor_tensor(ctxT[0:DH, hs, bs:bs + S],
                                        psa[0:DH, :], rec0[0:DH, :], OP.mult)
                nc.vector.tensor_tensor(ctxT[DH:P, hs, bs:bs + S],
                                        psb[DH:P, :], rec1[DH:P, :], OP.mult)
        ps_d1.close()
        ps_d0.close()
        ps_cb.close()
        ps_ca.close()
        ps_s.close()
        ph2.close()
        p_qkv.close()

        # ---- Phase 3: attention dense + residual + LN1 + transpose, per token
        # chunk (keeps PE fed: next chunk's matmuls overlap LN1/transposes) ----
        p_atok = _Pool(tc, "atok", 1)
        a_tok = p_atok.tile([P, TCH, D], F32, tag="a_tok")
        p_aT = _Pool(tc, "aT", 1)
        aT = p_aT.tile([P, DSUB, T], F32R, tag="aT")
        ph3 = _Pool(tc, "ph3", 2)
        ph3x = _Pool(tc, "ph3x", 3)
        ps3 = _Pool(tc, "ps3", 2, space="PSUM")
        ps4 = _Pool(tc, "ps4", 2, space="PSUM")
        ln1 = _Pool(tc, "ln1", 4)
        wor = wo_d.ap().rearrange("(ks p) m -> p ks m", p=P)
        wo_t = []
        for jh in range(NTH):
            wt = ph3.tile([P, DSUB, NT], F32R, tag="w_o")
            nc.gpsimd.dma_start(wt[:], wor[:, :, jh * NT:(jh + 1) * NT])
            wo_t.append(wt)
        for tc_i in range(TCH):
            xres = ph3x.tile([P, D], F32, tag="xres")
            nc.sync.dma_start(xres[:], x_d.ap()[tc_i * P:(tc_i + 1) * P, :])
            psa = ps3.tile([P, NT], F32, tag="ps_oA")
            psb = ps3.tile([P, NT], F32, tag="ps_oB")
            for ks in range(DSUB):
                lhs = ctxT[:, ks, tc_i * P:(tc_i + 1) * P]
                nc.tensor.matmul(psa[:], lhs, wo_t[0][:, ks, :],
                                 start=(ks == 0), stop=(ks == DSUB - 1))
                nc.tensor.matmul(psb[:], lhs, wo_t[1][:, ks, :],
                                 start=(ks == 0), stop=(ks == DSUB - 1))
            row = a_tok[:, tc_i, :]
            nc.vector.tensor_tensor(row[:, 0:NT], psa[:], bo_b[:, 0:NT], OP.add)
            nc.vector.tensor_tensor(row[:, NT:D], psb[:], bo_b[:, NT:D], OP.add)
            nc.vector.tensor_tensor(row, row, xres[:], OP.add)
            _layer_norm_rows(nc, ln1, row, g1_b, b1_b, eps_col, "ln1")
            for ds in range(DSUB):
                pst = ps4.tile([P, P], F32, tag="pst4")
                nc.tensor.transpose(pst[:], row[:, ds * P:(ds + 1) * P], ident[:])
                nc.vector.tensor_copy(aT[:, ds, tc_i * P:(tc_i + 1) * P], pst[:])
        ln1.close()
        ps4.close()
        ps3.close()
        ph3x.close()
        ph3.close()

        # a_tok now becomes the output accumulator: out = a_tok + bo2 + FFN
        for tc_i in range(TCH):
            row = a_tok[:, tc_i, :]
            nc.vector.tensor_tensor(row, row, bo2_b[:], OP.add)

        # ---- Phase 4: FFN, f dimension processed in quarters; the final
        # quarter fuses LN2 + output store per token chunk ----
        NFR = 4
        FSH = F // NFR // P            # 8 subtiles per round
        p_int = _Pool(tc, "inter", 1)
        interT = p_int.tile([P, FSH, T], F32R, tag="interT")
        ph5 = _Pool(tc, "ph5", 2)
        ph5w = _Pool(tc, "ph5w", 3)
        ps5i = _Pool(tc, "ps5i", 2, space="PSUM")
        ps5o = _Pool(tc, "ps5o", 2, space="PSUM")
        wo2r = wo2_d.ap().rearrange("(ks p) m -> p ks m", p=P)
        wir = wi_d.ap().rearrange("(ks p) m -> p ks m", p=P)
        for fh in range(NFR):
            # intermediate: interT = gelu(aT.T @ Wi + bi)^T   (feature-major)
            for fs in range(FSH):
                fchunk = fh * FSH + fs
                wt = ph5w.tile([P, DSUB, P], F32R, tag="w_i")
                nc.gpsimd.dma_start(wt[:], wir[:, :, fchunk * P:(fchunk + 1) * P])
                psa = ps5i.tile([P, NT], F32, tag="ps_iA")
                psb = ps5i.tile([P, NT], F32, tag="ps_iB")
                for ks in range(DSUB):
                    nc.tensor.matmul(psa[:], wt[:, ks, :], aT[:, ks, 0:NT],
                                     start=(ks == 0), stop=(ks == DSUB - 1))
                    nc.tensor.matmul(psb[:], wt[:, ks, :], aT[:, ks, NT:T],
                                     start=(ks == 0), stop=(ks == DSUB - 1))
                nc.scalar.activation(interT[:, fs, 0:NT], psa[:], AF.Gelu,
                                     bias=bi_col[:, fchunk:fchunk + 1], scale=1.0)
                nc.scalar.activation(interT[:, fs, NT:T], psb[:], AF.Gelu,
                                     bias=bi_col[:, fchunk:fchunk + 1], scale=1.0)
            # output: accumulate interT.T @ Wo2 into a_tok (token-major)
            w2_t = []
            for jh in range(NTH):
                wt2 = ph5.tile([P, FSH, NT], F32R, tag="w_o2")
                nc.gpsimd.dma_start(
                    wt2[:], wo2r[:, fh * FSH:(fh + 1) * FSH, jh * NT:(jh + 1) * NT])
                w2_t.append(wt2)
            for tc_i in range(TCH):
                psa = ps5o.tile([P, NT], F32, tag="ps_o2A")
                psb = ps5o.tile([P, NT], F32, tag="ps_o2B")
                for ks in range(FSH):
                    lhs = interT[:, ks, tc_i * P:(tc_i + 1) * P]
                    nc.tensor.matmul(psa[:], lhs, w2_t[0][:, ks, :],
                                     start=(ks == 0), stop=(ks == FSH - 1))
                    nc.tensor.matmul(psb[:], lhs, w2_t[1][:, ks, :],
                                     start=(ks == 0), stop=(ks == FSH - 1))
                row = a_tok[:, tc_i, :]
                nc.vector.tensor_tensor(row[:, 0:NT], row[:, 0:NT], psa[:], OP.add)
                nc.vector.tensor_tensor(row[:, NT:D], row[:, NT:D], psb[:], OP.add)
                if fh == NFR - 1:
                    _layer_norm_rows(nc, ph5, row, g2_b, b2_b, eps_col, "ln2")
                    nc.sync.dma_start(y_d.ap()[tc_i * P:(tc_i + 1) * P, :], row)
        ps5o.close()
        ps5i.close()
        ph5w.close()
        ph5.close()
        p_int.close()
        p_aT.close()
        p_atok.close()
        p_fm.close()
    const.close()


def build_nc(loop_n=None):
    nc = bacc.Bacc("TRN2", num_devices=NCORES)
    with tile.TileContext(nc) as tc:
        build_bert_layer(tc, loop_n=loop_n)
    nc.compile()
    return nc


_CACHE = {}


def make_in_maps(hidden_states, attention_mask, Wq, bq, Wk, bk, Wv, bv, Wo, bo,
                 ln1_g, ln1_b, Wi, bi, Wo2, bo2, ln2_g, ln2_b):
    common = {
        "Wq": np.asarray(Wq, np.float32), "bq": np.asarray(bq, np.float32),
        "Wk": np.asarray(Wk, np.float32), "bk": np.asarray(bk, np.float32),
        "Wv": np.asarray(Wv, np.float32), "bv": np.asarray(bv, np.float32),
        "Wo": np.asarray(Wo, np.float32), "bo": np.asarray(bo, np.float32),
        "ln1_g": np.asarray(ln1_g, np.float32), "ln1_b": np.asarray(ln1_b, np.float32),
        "Wi": np.asarray(Wi, np.float32), "bi": np.asarray(bi, np.float32),
        "Wo2": np.asarray(Wo2, np.float32), "bo2": np.asarray(bo2, np.float32),
        "ln2_g": np.asarray(ln2_g, np.float32), "ln2_b": np.asarray(ln2_b, np.float32),
    }
    x = np.asarray(hidden_states, np.float32).reshape(B, S, D)
    m = np.asarray(attention_mask, np.float32).reshape(B, S)
    in_maps = []
    for c in range(NCORES):
        in_maps.append({
            "x": np.ascontiguousarray(x[c * BPC:(c + 1) * BPC].reshape(T, D)),
            "mask": np.ascontiguousarray(m[c * BPC:(c + 1) * BPC]),
            **common,
        })
    return in_maps


def kernel(**inputs) -> np.ndarray:
    if "nc" not in _CACHE:
        _CACHE["nc"] = build_nc()
    nc = _CACHE["nc"]
    in_maps = make_in_maps(**inputs)
    res = run_bass_kernel_spmd(nc, in_maps, core_ids=list(range(NCORES)))
    out = np.concatenate([res.results[c]["y"] for c in range(NCORES)], axis=0)
    return out.reshape(B, S, D)


# revision 27
# speedup vs baseline: 1.2055x; 1.2055x over previous
"""BERT encoder layer on 8 TRN2 NeuronCores (Bass/Tile), data-parallel over batch.

Full inputs: hidden_states [16, 512, 1024], attention_mask [16, 512], weights.
Each core processes 2 batch items (1024 tokens). Weights are replicated; no
collectives. All matmuls run in float32r (~1e-4 rel err vs the fp32 reference).

Layout strategy: activations flow feature-major ("T" suffix = [feature, token])
through the attention block, so stored [in,out] weight matrices are directly
the matmul stationary operand (out = lhsT.T @ rhs) and no weight transposes are
needed. Scores are computed transposed [kt, qt] so the attention mask is a
per-partition bias on the Exp activation, and softmax normalization is deferred:
v is augmented with a ones column so the ctx matmul simultaneously produces the
softmax denominator. LayerNorms run token-major (bn_stats over the free dim);
the two layout flips (input x, and attn_out before the FFN) use PE transposes.

Perf-critical detail (measured on HW): an fp32r matmul whose stationary operand
(lhsT) changes every instruction runs at ~2-5x the ideal 512-cycle spacing,
while back-to-back matmuls REUSING the same lhsT run at ~1.05x ideal. All dense
projections therefore interleave two PSUM accumulation groups so each lhsT tile
is consumed by two consecutive matmuls (the two 512-token/feature halves).
"""

import contextlib

import numpy as np

import concourse.bass as bass
import concourse.mybir as mybir
import concourse.tile as tile
from concourse import bacc
from concourse.bass_utils import run_bass_kernel_spmd
from concourse.masks import make_identity

F32 = mybir.dt.float32
F32R = mybir.dt.float32r
AF = mybir.ActivationFunctionType
OP = mybir.AluOpType

B, S, D, H, F = 16, 512, 1024, 16, 4096
DH = D // H                      # 64
LN_EPS = 1e-12
NCORES = 8
BPC = B // NCORES                # 2 batch items per core
T = BPC * S                      # 1024 tokens per core
P = 128
DSUB = D // P                    # 8
TCH = T // P                     # 8 token chunks
SCH = S // P                     # 4 key chunks per batch item
NT = 512                         # matmul moving-dim tile
NTH = T // NT                    # 2 token halves


class _Pool:
    """Manually-scoped tile pool (pools must close in LIFO stack order)."""

    def __init__(self, tc, name, bufs, space="SBUF"):
        self._cm = tc.tile_pool(name=name, bufs=bufs, space=space)
        self.pool = self._cm.__enter__()

    def tile(self, *a, **k):
        if "name" not in k:
            k["name"] = k.get("tag", "t")
        return self.pool.tile(*a, **k)

    def close(self):
        self._cm.__exit__(None, None, None)


def _load_bias_cols(nc, pool, dram_vec, n_sub, tag, scale=None):
    """[n_sub*P] DRAM vector -> [P, n_sub] SBUF (feature d -> [d%P, d//P])."""
    col = pool.tile([P, n_sub], F32, tag=tag)
    nc.sync.dma_start(col[:], dram_vec.rearrange("(c p) -> p c", p=P))
    if scale is not None:
        nc.vector.tensor_scalar_mul(col[:], col[:], scale)
    return col


def _load_bcast(nc, pool, dram_vec, tag):
    """[D] DRAM vector -> [P, D] SBUF, replicated across partitions."""
    t = pool.tile([P, dram_vec.shape[0]], F32, tag=tag)
    src = bass.AP(tensor=dram_vec.tensor, offset=dram_vec.offset,
                  ap=[[0, P]] + list(dram_vec.ap))
    nc.sync.dma_start(out=t[:], in_=src)
    return t


def _layer_norm_rows(nc, pool, row, g_b, b_b, eps_col, tag):
    """In-place LayerNorm of [P, D] token rows over the free (feature) dim."""
    st = pool.tile([P, 2, 6], F32, tag=f"{tag}_st")
    nc.vector.bn_stats(st[:, 0, :], row[:, 0:512])
    nc.vector.bn_stats(st[:, 1, :], row[:, 512:1024])
    mv = pool.tile([P, 2], F32, tag=f"{tag}_mv")
    nc.vector.bn_aggr(mv[:], st[:])
    std = pool.tile([P, 1], F32, tag=f"{tag}_std")
    nc.scalar.activation(std[:], mv[:, 1:2], AF.Sqrt, bias=eps_col[:], scale=1.0)
    nc.vector.reciprocal(std[:], std[:])
    nc.vector.tensor_scalar(row, row, mv[:, 0:1], std[:], OP.subtract, OP.mult)
    nc.vector.tensor_tensor(row, row, g_b[:], OP.mult)
    nc.vector.tensor_tensor(row, row, b_b[:], OP.add)


def build_bert_layer(tc, loop_n=None):
    nc = tc.nc
    dt = nc.dram_tensor
    x_d = dt("x", [T, D], F32, kind="ExternalInput")
    mask_d = dt("mask", [BPC, S], F32, kind="ExternalInput")
    wq_d = dt("Wq", [D, D], F32, kind="ExternalInput")
    bq_d = dt("bq", [D], F32, kind="ExternalInput")
    wk_d = dt("Wk", [D, D], F32, kind="ExternalInput")
    bk_d = dt("bk", [D], F32, kind="ExternalInput")
    wv_d = dt("Wv", [D, D], F32, kind="ExternalInput")
    bv_d = dt("bv", [D], F32, kind="ExternalInput")
    wo_d = dt("Wo", [D, D], F32, kind="ExternalInput")
    bo_d = dt("bo", [D], F32, kind="ExternalInput")
    g1_d = dt("ln1_g", [D], F32, kind="ExternalInput")
    b1_d = dt("ln1_b", [D], F32, kind="ExternalInput")
    wi_d = dt("Wi", [D, F], F32, kind="ExternalInput")
    bi_d = dt("bi", [F], F32, kind="ExternalInput")
    wo2_d = dt("Wo2", [F, D], F32, kind="ExternalInput")
    bo2_d = dt("bo2", [D], F32, kind="ExternalInput")
    g2_d = dt("ln2_g", [D], F32, kind="ExternalInput")
    b2_d = dt("ln2_b", [D], F32, kind="ExternalInput")
    y_d = dt("y", [T, D], F32, kind="ExternalOutput")

    const = _Pool(tc, "const", 1)
    ident = const.tile([P, P], F32, tag="ident")
    make_identity(nc, ident)
    ident_r = const.tile([P, P], F32R, tag="ident_r")
    nc.vector.tensor_copy(ident_r[:], ident[:])
    zeros_f = const.tile([P, S], F32, tag="zeros_f")
    nc.vector.memset(zeros_f[:], 0.0)
    zeros_r = const.tile([P, S], F32R, tag="zeros_r")
    nc.vector.tensor_copy(zeros_r[:], zeros_f[:])
    ones_col = const.tile([P, 1], F32, tag="ones_col")
    nc.vector.memset(ones_col[:], 1.0)
    ones_r = const.tile([P, 1], F32R, tag="ones_r")
    nc.vector.tensor_copy(ones_r[:], ones_col[:])
    eps_col = const.tile([P, 1], F32, tag="eps")
    nc.vector.memset(eps_col, LN_EPS)
    # per-feature bias columns for feature-major stages (bias = per-partition)
    bqs_col = _load_bias_cols(nc, const, bq_d.ap(), DSUB, "bqs", scale=1.0 / np.sqrt(DH))
    bk_col = _load_bias_cols(nc, const, bk_d.ap(), DSUB, "bk")
    bi_col = _load_bias_cols(nc, const, bi_d.ap(), F // P, "bi")
    # per-feature vectors broadcast across partitions for token-major stages
    bv_b = _load_bcast(nc, const, bv_d.ap(), "bv_b")
    bo_b = _load_bcast(nc, const, bo_d.ap(), "bo_b")
    g1_b = _load_bcast(nc, const, g1_d.ap(), "g1_b")
    b1_b = _load_bcast(nc, const, b1_d.ap(), "b1_b")
    bo2_b = _load_bcast(nc, const, bo2_d.ap(), "bo2_b")
    g2_b = _load_bcast(nc, const, g2_d.ap(), "g2_b")
    b2_b = _load_bcast(nc, const, b2_d.ap(), "b2_b")
    # mask[b, kt] -> [kt%P, b, kt//P] so exp can take it as per-partition bias
    mask_sb = const.tile([P, BPC, SCH], F32, tag="mask")
    for b in range(BPC):
        nc.sync.dma_start(mask_sb[:, b, :],
                          mask_d.ap()[b].rearrange("(c p) -> p c", p=P))

    loop_cm = tc.For_i(0, loop_n, 1) if loop_n else contextlib.nullcontext()
    with loop_cm:
        # fm pool: one [P, DSUB, T] f32r slot shared by Xt (ph0-1), ctxT (ph2-3)
        p_fm = _Pool(tc, "fm", 1)

        # ---- Phase 0: load x token-major, PE-transpose to feature-major Xt ----
        xt = p_fm.tile([P, DSUB, T], F32R, tag="fm")  # Xt[p, ds, t] = x[t, ds*P+p]
        ph0 = _Pool(tc, "ph0", 3)
        ps0 = _Pool(tc, "ps0", 2, space="PSUM")
        for tc_i in range(TCH):
            xtok = ph0.tile([P, D], F32R, tag="xtok")
            nc.gpsimd.dma_start(xtok[:], x_d.ap()[tc_i * P:(tc_i + 1) * P, :])
            for ds in range(DSUB):
                pst = ps0.tile([P, P], F32R, tag="pst")
                nc.tensor.transpose(pst[:], xtok[:, ds * P:(ds + 1) * P], ident_r[:])
                nc.vector.tensor_copy(xt[:, ds, tc_i * P:(tc_i + 1) * P], pst[:])
        ps0.close()
        ph0.close()

        # ---- Phase 1: QKV projections (each lhsT reused by 2 matmuls) ----
        p_qkv = _Pool(tc, "qkv", 1)
        qT = p_qkv.tile([P, DSUB, T], F32R, tag="qT")
        kT = p_qkv.tile([P, DSUB, T], F32R, tag="kT")
        v_sb = p_qkv.tile([P, TCH, D], F32R, tag="v_sb")
        ph1 = _Pool(tc, "ph1", 2)
        ph1w = _Pool(tc, "ph1w", 3)
        ps1 = _Pool(tc, "ps1", 2, space="PSUM")

        for name, w_dram, dst, bias_col, scale in (
            ("q", wq_d, qT, bqs_col, 1.0 / np.sqrt(DH)),
            ("k", wk_d, kT, bk_col, 1.0),
        ):
            wr = w_dram.ap().rearrange("(ks p) m -> p ks m", p=P)
            for mo in range(DSUB):  # output-feature chunk
                wt = ph1w.tile([P, DSUB, P], F32R, tag="w_qkv")
                nc.gpsimd.dma_start(wt[:], wr[:, :, mo * P:(mo + 1) * P])
                psa = ps1.tile([P, NT], F32, tag="ps_qA")
                psb = ps1.tile([P, NT], F32, tag="ps_qB")
                for ks in range(DSUB):
                    nc.tensor.matmul(psa[:], wt[:, ks, :], xt[:, ks, 0:NT],
                                     start=(ks == 0), stop=(ks == DSUB - 1))
                    nc.tensor.matmul(psb[:], wt[:, ks, :], xt[:, ks, NT:T],
                                     start=(ks == 0), stop=(ks == DSUB - 1))
                # dst = scale*psum + scale*bias  (scale folded into bias col)
                nc.scalar.activation(dst[:, mo, 0:NT], psa[:], AF.Identity,
                                     bias=bias_col[:, mo:mo + 1], scale=scale)
                nc.scalar.activation(dst[:, mo, NT:T], psb[:], AF.Identity,
                                     bias=bias_col[:, mo:mo + 1], scale=scale)

        # v token-major: lhsT = Xt token chunk (reused for both Wv halves)
        wvr = wv_d.ap().rearrange("(ks p) m -> p ks m", p=P)
        wv_t = []
        for jh in range(NTH):
            wvt = ph1.tile([P, DSUB, NT], F32R, tag="w_v")
            nc.gpsimd.dma_start(wvt[:], wvr[:, :, jh * NT:(jh + 1) * NT])
            wv_t.append(wvt)
        for tc_i in range(TCH):
            psa = ps1.tile([P, NT], F32, tag="ps_vA")
            psb = ps1.tile([P, NT], F32, tag="ps_vB")
            for ks in range(DSUB):
                lhs = xt[:, ks, tc_i * P:(tc_i + 1) * P]
                nc.tensor.matmul(psa[:], lhs, wv_t[0][:, ks, :],
                                 start=(ks == 0), stop=(ks == DSUB - 1))
                nc.tensor.matmul(psb[:], lhs, wv_t[1][:, ks, :],
                                 start=(ks == 0), stop=(ks == DSUB - 1))
            for jh, ps in ((0, psa), (1, psb)):
                nc.vector.tensor_tensor(
                    v_sb[:, tc_i, jh * NT:(jh + 1) * NT], ps[:],
                    bv_b[:, jh * NT:(jh + 1) * NT], OP.add)
        ps1.close()
        ph1w.close()
        ph1.close()

        # ---- Phase 2: attention ----
        # Scores use K=128: lhsT is a full kT subtile (two heads stacked) and
        # the rhs is q zero-padded outside its head rows, so each kT chunk is a
        # stationary operand REUSED by both heads of the pair (fp32r matmuls
        # with a fresh lhsT each are ~5x slower than reusing ones). The ctx
        # matmuls reuse a [v_even | v_odd] pair block the same way; garbage
        # rows are simply not read back. Softmax denominators come from cheap
        # 2-column one-hot matmuls accumulated per pair.
        ctxT = p_fm.tile([P, DSUB, T], F32R, tag="fm")  # reuses the Xt slot
        ph2 = _Pool(tc, "ph2", 2)
        ps_s = _Pool(tc, "ps_s", 2, space="PSUM")
        ps_ca = _Pool(tc, "ps_ca", 2, space="PSUM")
        ps_cb = _Pool(tc, "ps_cb", 2, space="PSUM")
        ps_d0 = _Pool(tc, "ps_d0", 1, space="PSUM")
        ps_d1 = _Pool(tc, "ps_d1", 1, space="PSUM")
        for b in range(BPC):
            bs = b * S
            for hs in range(DSUB):
                # zero-padded q tiles for the even/odd head of this pair
                pad0 = ph2.tile([P, S], F32R, tag="pad0", bufs=2)
                nc.scalar.copy(pad0[DH:P, :], zeros_r[DH:P, :])
                nc.vector.tensor_copy(pad0[0:DH, :], qT[0:DH, hs, bs:bs + S])
                pad1 = ph2.tile([P, S], F32R, tag="pad1", bufs=2)
                nc.scalar.copy(pad1[0:DH, :], zeros_r[0:DH, :])
                nc.vector.tensor_copy(pad1[DH:P, :], qT[DH:P, hs, bs:bs + S])
                exps = {}
                for c in range(SCH):
                    lhs = kT[:, hs, bs + c * P:bs + (c + 1) * P]
                    for par, pad in ((0, pad0), (1, pad1)):
                        t = ps_s.tile([P, S], F32, tag="pss")
                        nc.tensor.matmul(t[:], lhs, pad[:], start=True, stop=True)
                        e = ph2.tile([P, S], F32R, tag="expT", bufs=7)
                        nc.scalar.activation(e[:], t[:], AF.Exp,
                                             bias=mask_sb[:, b, c:c + 1], scale=1.0)
                        exps[par, c] = e
                psa = ps_ca.tile([P, S], F32, tag="ps_cA")
                psb = ps_cb.tile([P, S], F32, tag="ps_cB")
                for c in range(SCH):
                    vp = v_sb[:, b * SCH + c, hs * P:(hs + 1) * P]
                    nc.tensor.matmul(psa[:], vp, exps[0, c][:],
                                     start=(c == 0), stop=(c == SCH - 1))
                    nc.tensor.matmul(psb[:], vp, exps[1, c][:],
                                     start=(c == 0), stop=(c == SCH - 1))
                # sum the exp chunks on DVE so the denominator needs only
                # one N=512 matmul per parity instead of four
                psd0 = ps_d0.tile([1, S], F32, tag="ps_den0")
                psd1 = ps_d1.tile([1, S], F32, tag="ps_den1")
                for par, psd in ((0, psd0), (1, psd1)):
                    s01 = ph2.tile([P, S], F32R, tag="esumt", bufs=4)
                    nc.vector.tensor_tensor(s01[:], exps[par, 0][:],
                                            exps[par, 1][:], OP.add)
                    s23 = ph2.tile([P, S], F32R, tag="esumt", bufs=4)
                    nc.vector.tensor_tensor(s23[:], exps[par, 2][:],
                                            exps[par, 3][:], OP.add)
                    esum = ph2.tile([P, S], F32R, tag="esumt", bufs=4)
                    nc.vector.tensor_tensor(esum[:], s01[:], s23[:], OP.add)
                    nc.tensor.matmul(psd[:], ones_r[:], esum[:],
                                     start=True, stop=True)
                den0 = ph2.tile([1, S], F32, tag="den0")
                nc.scalar.copy(den0[:], psd0[:])
                den1 = ph2.tile([1, S], F32, tag="den1")
                nc.scalar.copy(den1[:], psd1[:])
                rec0 = ph2.tile([DH, S], F32, tag="rec0", bufs=2)
                nc.gpsimd.partition_broadcast(rec0[:], den0[:])
                rec1 = ph2.tile([P, S], F32, tag="rec1", bufs=2)
                nc.gpsimd.partition_broadcast(rec1[:], den1[:])
                nc.vector.reciprocal(rec0[:, :], rec0[:, :])
                nc.vector.reciprocal(rec1[DH:P, :], rec1[DH:P, :])
                nc.vector.tensor_tensor(ctxT[0:DH, hs, bs:bs + S],
                                        psa[0:DH, :], rec0[:, :], OP.mult)
                nc.vector.tensor_tensor(ctxT[DH:P, hs, bs:bs + S],
                                        psb[DH:P, :], rec1[DH:P, :], OP.mult)
        ps_d1.close()
        ps_d0.close()
        ps_cb.close()
        ps_ca.close()
        ps_s.close()
        ph2.close()
        p_qkv.close()

        # ---- Phase 3: attention dense + residual + LN1 + transpose, per token
        # chunk (keeps PE fed: next chunk's matmuls overlap LN1/transposes) ----
        p_atok = _Pool(tc, "atok", 1)
        a_tok = p_atok.tile([P, TCH, D], F32, tag="a_tok")
        p_aT = _Pool(tc, "aT", 1)
        aT = p_aT.tile([P, DSUB, T], F32R, tag="aT")
        ph3 = _Pool(tc, "ph3", 2)
        ph3x = _Pool(tc, "ph3x", 3)
        ps3 = _Pool(tc, "ps3", 2, space="PSUM")
        ps4 = _Pool(tc, "ps4", 2, space="PSUM")
        ln1 = _Pool(tc, "ln1", 4)
        wor = wo_d.ap().rearrange("(ks p) m -> p ks m", p=P)
        wo_t = []
        for jh in range(NTH):
            wt = ph3.tile([P, DSUB, NT], F32R, tag="w_o")
            nc.gpsimd.dma_start(wt[:], wor[:, :, jh * NT:(jh + 1) * NT])
            wo_t.append(wt)
        for tc_i in range(TCH):
            xres = ph3x.tile([P, D], F32, tag="xres")
            nc.sync.dma_start(xres[:], x_d.ap()[tc_i * P:(tc_i + 1) * P, :])
            psa = ps3.tile([P, NT], F32, tag="ps_oA")
            psb = ps3.tile([P, NT], F32, tag="ps_oB")
            for ks in range(DSUB):
                lhs = ctxT[:, ks, tc_i * P:(tc_i + 1) * P]
                nc.tensor.matmul(psa[:], lhs, wo_t[0][:, ks, :],
                                 start=(ks == 0), stop=(ks == DSUB - 1))
                nc.tensor.matmul(psb[:], lhs, wo_t[1][:, ks, :],
                                 start=(ks == 0), stop=(ks == DSUB - 1))
            row = a_tok[:, tc_i, :]
            nc.vector.tensor_tensor(row[:, 0:NT], psa[:], bo_b[:, 0:NT], OP.add)
            nc.vector.tensor_tensor(row[:, NT:D], psb[:], bo_b[:, NT:D], OP.add)
            nc.vector.tensor_tensor(row, row, xres[:], OP.add)
            _layer_norm_rows(nc, ln1, row, g1_b, b1_b, eps_col, "ln1")
            for ds in range(DSUB):
                pst = ps4.tile([P, P], F32, tag="pst4")
                nc.tensor.transpose(pst[:], row[:, ds * P:(ds + 1) * P], ident[:])
                nc.vector.tensor_copy(aT[:, ds, tc_i * P:(tc_i + 1) * P], pst[:])
        ln1.close()
        ps4.close()
        ps3.close()
        ph3x.close()
        ph3.close()

        # a_tok now becomes the output accumulator: out = a_tok + bo2 + FFN
        for tc_i in range(TCH):
            row = a_tok[:, tc_i, :]
            nc.vector.tensor_tensor(row, row, bo2_b[:], OP.add)

        # ---- Phase 4: FFN, f dimension processed in quarters; the final
        # quarter fuses LN2 + output store per token chunk ----
        NFR = 4
        FSH = F // NFR // P            # 8 subtiles per round
        p_int = _Pool(tc, "inter", 1)
        interT = p_int.tile([P, FSH, T], F32R, tag="interT")
        ph5 = _Pool(tc, "ph5", 2)
        ph5w = _Pool(tc, "ph5w", 3)
        ps5i = _Pool(tc, "ps5i", 2, space="PSUM")
        ps5o = _Pool(tc, "ps5o", 2, space="PSUM")
        wo2r = wo2_d.ap().rearrange("(ks p) m -> p ks m", p=P)
        wir = wi_d.ap().rearrange("(ks p) m -> p ks m", p=P)
        for fh in range(NFR):
            # intermediate: interT = gelu(aT.T @ Wi + bi)^T   (feature-major)
            for fs in range(FSH):
                fchunk = fh * FSH + fs
                wt = ph5w.tile([P, DSUB, P], F32R, tag="w_i")
                nc.gpsimd.dma_start(wt[:], wir[:, :, fchunk * P:(fchunk + 1) * P])
                psa = ps5i.tile([P, NT], F32, tag="ps_iA")
                psb = ps5i.tile([P, NT], F32, tag="ps_iB")
                for ks in range(DSUB):
                    nc.tensor.matmul(psa[:], wt[:, ks, :], aT[:, ks, 0:NT],
                                     start=(ks == 0), stop=(ks == DSUB - 1))
                    nc.tensor.matmul(psb[:], wt[:, ks, :], aT[:, ks, NT:T],
                                     start=(ks == 0), stop=(ks == DSUB - 1))
                nc.scalar.activation(interT[:, fs, 0:NT], psa[:], AF.Gelu,
                                     bias=bi_col[:, fchunk:fchunk + 1], scale=1.0)
                nc.scalar.activation(interT[:, fs, NT:T], psb[:], AF.Gelu,
                                     bias=bi_col[:, fchunk:fchunk + 1], scale=1.0)
            # output: accumulate interT.T @ Wo2 into a_tok (token-major)
            w2_t = []
            for jh in range(NTH):
                wt2 = ph5.tile([P, FSH, NT], F32R, tag="w_o2")
                nc.gpsimd.dma_start(
                    wt2[:], wo2r[:, fh * FSH:(fh + 1) * FSH, jh * NT:(jh + 1) * NT])
                w2_t.append(wt2)
            for tc_i in range(TCH):
                psa = ps5o.tile([P, NT], F32, tag="ps_o2A")
                psb = ps5o.tile([P, NT], F32, tag="ps_o2B")
                for ks in range(FSH):
                    lhs = interT[:, ks, tc_i * P:(tc_i + 1) * P]
                    nc.tensor.matmul(psa[:], lhs, w2_t[0][:, ks, :],
                                     start=(ks == 0), stop=(ks == FSH - 1))
                    nc.tensor.matmul(psb[:], lhs, w2_t[1][:, ks, :],
                                     start=(ks == 0), stop=(ks == FSH - 1))
                row = a_tok[:, tc_i, :]
                nc.vector.tensor_tensor(row[:, 0:NT], row[:, 0:NT], psa[:], OP.add)
                nc.vector.tensor_tensor(row[:, NT:D], row[:, NT:D], psb[:], OP.add)
                if fh == NFR - 1:
                    _layer_norm_rows(nc, ph5, row, g2_b, b2_b, eps_col, "ln2")
                    nc.sync.dma_start(y_d.ap()[tc_i * P:(tc_i + 1) * P, :], row)
        ps5o.close()
        ps5i.close()
        ph5w.close()
        ph5.close()
        p_int.close()
        p_aT.close()
        p_atok.close()
        p_fm.close()
    const.close()


def build_nc(loop_n=None):
    nc = bacc.Bacc("TRN2", num_devices=NCORES)
    with tile.TileContext(nc) as tc:
        build_bert_layer(tc, loop_n=loop_n)
    nc.compile()
    return nc


_CACHE = {}


def make_in_maps(hidden_states, attention_mask, Wq, bq, Wk, bk, Wv, bv, Wo, bo,
                 ln1_g, ln1_b, Wi, bi, Wo2, bo2, ln2_g, ln2_b):
    common = {
        "Wq": np.asarray(Wq, np.float32), "bq": np.asarray(bq, np.float32),
        "Wk": np.asarray(Wk, np.float32), "bk": np.asarray(bk, np.float32),
        "Wv": np.asarray(Wv, np.float32), "bv": np.asarray(bv, np.float32),
        "Wo": np.asarray(Wo, np.float32), "bo": np.asarray(bo, np.float32),
        "ln1_g": np.asarray(ln1_g, np.float32), "ln1_b": np.asarray(ln1_b, np.float32),
        "Wi": np.asarray(Wi, np.float32), "bi": np.asarray(bi, np.float32),
        "Wo2": np.asarray(Wo2, np.float32), "bo2": np.asarray(bo2, np.float32),
        "ln2_g": np.asarray(ln2_g, np.float32), "ln2_b": np.asarray(ln2_b, np.float32),
    }
    x = np.asarray(hidden_states, np.float32).reshape(B, S, D)
    m = np.asarray(attention_mask, np.float32).reshape(B, S)
    in_maps = []
    for c in range(NCORES):
        in_maps.append({
            "x": np.ascontiguousarray(x[c * BPC:(c + 1) * BPC].reshape(T, D)),
            "mask": np.ascontiguousarray(m[c * BPC:(c + 1) * BPC]),
            **common,
        })
    return in_maps


def kernel(**inputs) -> np.ndarray:
    if "nc" not in _CACHE:
        _CACHE["nc"] = build_nc()
    nc = _CACHE["nc"]
    in_maps = make_in_maps(**inputs)
    res = run_bass_kernel_spmd(nc, in_maps, core_ids=list(range(NCORES)))
    out = np.concatenate([res.results[c]["y"] for c in range(NCORES)], axis=0)
    return out.reshape(B, S, D)


# revision 28
# speedup vs baseline: 1.2913x; 1.0712x over previous
"""BERT encoder layer on 8 TRN2 NeuronCores (Bass/Tile), data-parallel over batch.

Full inputs: hidden_states [16, 512, 1024], attention_mask [16, 512], weights.
Each core processes 2 batch items (1024 tokens). Weights are replicated; no
collectives. All matmuls run in float32r (~1e-4 rel err vs the fp32 reference).

Layout strategy: activations flow feature-major ("T" suffix = [feature, token])
through the attention block, so stored [in,out] weight matrices are directly
the matmul stationary operand (out = lhsT.T @ rhs) and no weight transposes are
needed. Scores are computed transposed [kt, qt] so the attention mask is a
per-partition bias on the Exp activation, and softmax normalization is deferred:
v is augmented with a ones column so the ctx matmul simultaneously produces the
softmax denominator. LayerNorms run token-major (bn_stats over the free dim);
the two layout flips (input x, and attn_out before the FFN) use PE transposes.

Perf-critical detail (measured on HW): an fp32r matmul whose stationary operand
(lhsT) changes every instruction runs at ~2-5x the ideal 512-cycle spacing,
while back-to-back matmuls REUSING the same lhsT run at ~1.05x ideal. All dense
projections therefore interleave two PSUM accumulation groups so each lhsT tile
is consumed by two consecutive matmuls (the two 512-token/feature halves).
"""

import contextlib

import numpy as np

import concourse.bass as bass
import concourse.mybir as mybir
import concourse.tile as tile
from concourse import bacc
from concourse.bass_utils import run_bass_kernel_spmd
from concourse.masks import make_identity

F32 = mybir.dt.float32
F32R = mybir.dt.float32r
AF = mybir.ActivationFunctionType
OP = mybir.AluOpType

B, S, D, H, F = 16, 512, 1024, 16, 4096
DH = D // H                      # 64
LN_EPS = 1e-12
NCORES = 8
BPC = B // NCORES                # 2 batch items per core
T = BPC * S                      # 1024 tokens per core
P = 128
DSUB = D // P                    # 8
TCH = T // P                     # 8 token chunks
SCH = S // P                     # 4 key chunks per batch item
NT = 512                         # matmul moving-dim tile
NTH = T // NT                    # 2 token halves


class _Pool:
    """Manually-scoped tile pool (pools must close in LIFO stack order)."""

    def __init__(self, tc, name, bufs, space="SBUF"):
        self._cm = tc.tile_pool(name=name, bufs=bufs, space=space)
        self.pool = self._cm.__enter__()

    def tile(self, *a, **k):
        if "name" not in k:
            k["name"] = k.get("tag", "t")
        return self.pool.tile(*a, **k)

    def close(self):
        self._cm.__exit__(None, None, None)


def _load_bias_cols(nc, pool, dram_vec, n_sub, tag, scale=None):
    """[n_sub*P] DRAM vector -> [P, n_sub] SBUF (feature d -> [d%P, d//P])."""
    col = pool.tile([P, n_sub], F32, tag=tag)
    nc.sync.dma_start(col[:], dram_vec.rearrange("(c p) -> p c", p=P))
    if scale is not None:
        nc.vector.tensor_scalar_mul(col[:], col[:], scale)
    return col


def _load_bcast(nc, pool, dram_vec, tag):
    """[D] DRAM vector -> [P, D] SBUF, replicated across partitions."""
    t = pool.tile([P, dram_vec.shape[0]], F32, tag=tag)
    src = bass.AP(tensor=dram_vec.tensor, offset=dram_vec.offset,
                  ap=[[0, P]] + list(dram_vec.ap))
    nc.sync.dma_start(out=t[:], in_=src)
    return t


def _layer_norm_rows(nc, pool, row, g_b, b_b, eps_col, tag):
    """In-place LayerNorm of [P, D] token rows over the free (feature) dim."""
    st = pool.tile([P, 2, 6], F32, tag=f"{tag}_st")
    nc.vector.bn_stats(st[:, 0, :], row[:, 0:512])
    nc.vector.bn_stats(st[:, 1, :], row[:, 512:1024])
    mv = pool.tile([P, 2], F32, tag=f"{tag}_mv")
    nc.vector.bn_aggr(mv[:], st[:])
    std = pool.tile([P, 1], F32, tag=f"{tag}_std")
    nc.scalar.activation(std[:], mv[:, 1:2], AF.Sqrt, bias=eps_col[:], scale=1.0)
    nc.vector.reciprocal(std[:], std[:])
    nc.vector.tensor_scalar(row, row, mv[:, 0:1], std[:], OP.subtract, OP.mult)
    nc.vector.tensor_tensor(row, row, g_b[:], OP.mult)
    nc.vector.tensor_tensor(row, row, b_b[:], OP.add)


def build_bert_layer(tc, loop_n=None):
    nc = tc.nc
    dt = nc.dram_tensor
    x_d = dt("x", [T, D], F32, kind="ExternalInput")
    mask_d = dt("mask", [BPC, S], F32, kind="ExternalInput")
    wq_d = dt("Wq", [D, D], F32, kind="ExternalInput")
    bq_d = dt("bq", [D], F32, kind="ExternalInput")
    wk_d = dt("Wk", [D, D], F32, kind="ExternalInput")
    bk_d = dt("bk", [D], F32, kind="ExternalInput")
    wv_d = dt("Wv", [D, D], F32, kind="ExternalInput")
    bv_d = dt("bv", [D], F32, kind="ExternalInput")
    wo_d = dt("Wo", [D, D], F32, kind="ExternalInput")
    bo_d = dt("bo", [D], F32, kind="ExternalInput")
    g1_d = dt("ln1_g", [D], F32, kind="ExternalInput")
    b1_d = dt("ln1_b", [D], F32, kind="ExternalInput")
    wi_d = dt("Wi", [D, F], F32, kind="ExternalInput")
    bi_d = dt("bi", [F], F32, kind="ExternalInput")
    wo2_d = dt("Wo2", [F, D], F32, kind="ExternalInput")
    bo2_d = dt("bo2", [D], F32, kind="ExternalInput")
    g2_d = dt("ln2_g", [D], F32, kind="ExternalInput")
    b2_d = dt("ln2_b", [D], F32, kind="ExternalInput")
    y_d = dt("y", [T, D], F32, kind="ExternalOutput")

    const = _Pool(tc, "const", 1)
    ident = const.tile([P, P], F32, tag="ident")
    make_identity(nc, ident)
    ident_r = const.tile([P, P], F32R, tag="ident_r")
    nc.vector.tensor_copy(ident_r[:], ident[:])
    zeros_f = const.tile([P, S], F32, tag="zeros_f")
    nc.vector.memset(zeros_f[:], 0.0)
    zeros_r = const.tile([P, S], F32R, tag="zeros_r")
    nc.vector.tensor_copy(zeros_r[:], zeros_f[:])
    ones_col = const.tile([P, 1], F32, tag="ones_col")
    nc.vector.memset(ones_col[:], 1.0)
    ones_r = const.tile([P, 1], F32R, tag="ones_r")
    nc.vector.tensor_copy(ones_r[:], ones_col[:])
    eps_col = const.tile([P, 1], F32, tag="eps")
    nc.vector.memset(eps_col, LN_EPS)
    # per-feature bias columns for feature-major stages (bias = per-partition)
    bqs_col = _load_bias_cols(nc, const, bq_d.ap(), DSUB, "bqs", scale=1.0 / np.sqrt(DH))
    bk_col = _load_bias_cols(nc, const, bk_d.ap(), DSUB, "bk")
    bi_col = _load_bias_cols(nc, const, bi_d.ap(), F // P, "bi")
    # per-feature vectors broadcast across partitions for token-major stages
    bv_b = _load_bcast(nc, const, bv_d.ap(), "bv_b")
    bo_b = _load_bcast(nc, const, bo_d.ap(), "bo_b")
    g1_b = _load_bcast(nc, const, g1_d.ap(), "g1_b")
    b1_b = _load_bcast(nc, const, b1_d.ap(), "b1_b")
    bo2_b = _load_bcast(nc, const, bo2_d.ap(), "bo2_b")
    g2_b = _load_bcast(nc, const, g2_d.ap(), "g2_b")
    b2_b = _load_bcast(nc, const, b2_d.ap(), "b2_b")
    # mask[b, kt] -> [kt%P, b, kt//P] so exp can take it as per-partition bias
    mask_sb = const.tile([P, BPC, SCH], F32, tag="mask")
    for b in range(BPC):
        nc.sync.dma_start(mask_sb[:, b, :],
                          mask_d.ap()[b].rearrange("(c p) -> p c", p=P))

    loop_cm = tc.For_i(0, loop_n, 1) if loop_n else contextlib.nullcontext()
    with loop_cm:
        # fm pool: one [P, DSUB, T] f32r slot shared by Xt (ph0-1), ctxT (ph2-3)
        p_fm = _Pool(tc, "fm", 1)

        # ---- Phase 0: load x token-major, PE-transpose to feature-major Xt ----
        xt = p_fm.tile([P, DSUB, T], F32R, tag="fm")  # Xt[p, ds, t] = x[t, ds*P+p]
        ph0 = _Pool(tc, "ph0", 3)
        ps0 = _Pool(tc, "ps0", 2, space="PSUM")
        for tc_i in range(TCH):
            xtok = ph0.tile([P, D], F32R, tag="xtok")
            nc.gpsimd.dma_start(xtok[:], x_d.ap()[tc_i * P:(tc_i + 1) * P, :])
            for ds in range(DSUB):
                pst = ps0.tile([P, P], F32R, tag="pst")
                nc.tensor.transpose(pst[:], xtok[:, ds * P:(ds + 1) * P], ident_r[:])
                nc.vector.tensor_copy(xt[:, ds, tc_i * P:(tc_i + 1) * P], pst[:])
        ps0.close()
        ph0.close()

        # ---- Phase 1: QKV projections (each lhsT reused by 2 matmuls) ----
        p_qkv = _Pool(tc, "qkv", 1)
        qT = p_qkv.tile([P, DSUB, T], F32R, tag="qT")
        kT = p_qkv.tile([P, DSUB, T], F32R, tag="kT")
        v_sb = p_qkv.tile([P, TCH, D], F32R, tag="v_sb")
        ph1 = _Pool(tc, "ph1", 2)
        ph1w = _Pool(tc, "ph1w", 3)
        ps1 = _Pool(tc, "ps1", 2, space="PSUM")

        for name, w_dram, dst, bias_col, scale in (
            ("q", wq_d, qT, bqs_col, 1.0 / np.sqrt(DH)),
            ("k", wk_d, kT, bk_col, 1.0),
        ):
            wr = w_dram.ap().rearrange("(ks p) m -> p ks m", p=P)
            for mo in range(DSUB):  # output-feature chunk
                wt = ph1w.tile([P, DSUB, P], F32R, tag="w_qkv")
                nc.gpsimd.dma_start(wt[:], wr[:, :, mo * P:(mo + 1) * P])
                psa = ps1.tile([P, NT], F32, tag="ps_qA")
                psb = ps1.tile([P, NT], F32, tag="ps_qB")
                for ks in range(DSUB):
                    nc.tensor.matmul(psa[:], wt[:, ks, :], xt[:, ks, 0:NT],
                                     start=(ks == 0), stop=(ks == DSUB - 1))
                    nc.tensor.matmul(psb[:], wt[:, ks, :], xt[:, ks, NT:T],
                                     start=(ks == 0), stop=(ks == DSUB - 1))
                # dst = scale*psum + scale*bias  (scale folded into bias col)
                nc.scalar.activation(dst[:, mo, 0:NT], psa[:], AF.Identity,
                                     bias=bias_col[:, mo:mo + 1], scale=scale)
                nc.scalar.activation(dst[:, mo, NT:T], psb[:], AF.Identity,
                                     bias=bias_col[:, mo:mo + 1], scale=scale)

        # v token-major: lhsT = Xt token chunk (reused for both Wv halves)
        wvr = wv_d.ap().rearrange("(ks p) m -> p ks m", p=P)
        wv_t = []
        for jh in range(NTH):
            wvt = ph1.tile([P, DSUB, NT], F32R, tag="w_v")
            nc.gpsimd.dma_start(wvt[:], wvr[:, :, jh * NT:(jh + 1) * NT])
            wv_t.append(wvt)
        for tc_i in range(TCH):
            psa = ps1.tile([P, NT], F32, tag="ps_vA")
            psb = ps1.tile([P, NT], F32, tag="ps_vB")
            for ks in range(DSUB):
                lhs = xt[:, ks, tc_i * P:(tc_i + 1) * P]
                nc.tensor.matmul(psa[:], lhs, wv_t[0][:, ks, :],
                                 start=(ks == 0), stop=(ks == DSUB - 1))
                nc.tensor.matmul(psb[:], lhs, wv_t[1][:, ks, :],
                                 start=(ks == 0), stop=(ks == DSUB - 1))
            for jh, ps in ((0, psa), (1, psb)):
                nc.vector.tensor_tensor(
                    v_sb[:, tc_i, jh * NT:(jh + 1) * NT], ps[:],
                    bv_b[:, jh * NT:(jh + 1) * NT], OP.add)
        ps1.close()
        ph1w.close()
        ph1.close()

        # ---- Phase 2: attention ----
        # Scores use K=128: lhsT is a full kT subtile (two heads stacked) and
        # the rhs is q zero-padded outside its head rows, so each kT chunk is a
        # stationary operand REUSED by both heads of the pair (fp32r matmuls
        # with a fresh lhsT each are ~5x slower than reusing ones). The ctx
        # matmuls reuse a [v_even | v_odd] pair block the same way; garbage
        # rows are simply not read back. Softmax denominators come from cheap
        # 2-column one-hot matmuls accumulated per pair.
        ctxT = p_fm.tile([P, DSUB, T], F32R, tag="fm")  # reuses the Xt slot
        ph2 = _Pool(tc, "ph2", 2)
        ps_s = _Pool(tc, "ps_s", 2, space="PSUM")
        ps_ca = _Pool(tc, "ps_ca", 2, space="PSUM")
        ps_cb = _Pool(tc, "ps_cb", 2, space="PSUM")
        ps_d0 = _Pool(tc, "ps_d0", 1, space="PSUM")
        ps_d1 = _Pool(tc, "ps_d1", 1, space="PSUM")
        for b in range(BPC):
            bs = b * S
            for hs in range(DSUB):
                # zero-padded q tiles for the even/odd head of this pair
                pad0 = ph2.tile([P, S], F32R, tag="pad0", bufs=2)
                nc.scalar.copy(pad0[DH:P, :], zeros_r[DH:P, :])
                nc.vector.tensor_copy(pad0[0:DH, :], qT[0:DH, hs, bs:bs + S])
                pad1 = ph2.tile([P, S], F32R, tag="pad1", bufs=2)
                nc.scalar.copy(pad1[0:DH, :], zeros_r[0:DH, :])
                nc.vector.tensor_copy(pad1[DH:P, :], qT[DH:P, hs, bs:bs + S])
                exps = {}
                for c in range(SCH):
                    lhs = kT[:, hs, bs + c * P:bs + (c + 1) * P]
                    for par, pad in ((0, pad0), (1, pad1)):
                        t = ps_s.tile([P, S], F32, tag="pss")
                        nc.tensor.matmul(t[:], lhs, pad[:], start=True, stop=True)
                        e = ph2.tile([P, S], F32R, tag="expT", bufs=7)
                        nc.scalar.activation(e[:], t[:], AF.Exp,
                                             bias=mask_sb[:, b, c:c + 1], scale=1.0)
                        exps[par, c] = e
                psa = ps_ca.tile([P, S], F32, tag="ps_cA")
                psb = ps_cb.tile([P, S], F32, tag="ps_cB")
                for c in range(SCH):
                    vp = v_sb[:, b * SCH + c, hs * P:(hs + 1) * P]
                    nc.tensor.matmul(psa[:], vp, exps[0, c][:],
                                     start=(c == 0), stop=(c == SCH - 1))
                    nc.tensor.matmul(psb[:], vp, exps[1, c][:],
                                     start=(c == 0), stop=(c == SCH - 1))
                psd0 = ps_d0.tile([1, S], F32, tag="ps_den0")
                psd1 = ps_d1.tile([1, S], F32, tag="ps_den1")
                for c in range(SCH):
                    nc.tensor.matmul(psd0[:], ones_r[:], exps[0, c][:],
                                     start=(c == 0), stop=(c == SCH - 1))
                    nc.tensor.matmul(psd1[:], ones_r[:], exps[1, c][:],
                                     start=(c == 0), stop=(c == SCH - 1))
                den0 = ph2.tile([1, S], F32, tag="den0")
                nc.scalar.copy(den0[:], psd0[:])
                den1 = ph2.tile([1, S], F32, tag="den1")
                nc.scalar.copy(den1[:], psd1[:])
                rec0 = ph2.tile([DH, S], F32, tag="rec0", bufs=2)
                nc.gpsimd.partition_broadcast(rec0[:], den0[:])
                rec1 = ph2.tile([P, S], F32, tag="rec1", bufs=2)
                nc.gpsimd.partition_broadcast(rec1[:], den1[:])
                nc.vector.reciprocal(rec0[:, :], rec0[:, :])
                nc.vector.reciprocal(rec1[DH:P, :], rec1[DH:P, :])
                nc.vector.tensor_tensor(ctxT[0:DH, hs, bs:bs + S],
                                        psa[0:DH, :], rec0[:, :], OP.mult)
                nc.vector.tensor_tensor(ctxT[DH:P, hs, bs:bs + S],
                                        psb[DH:P, :], rec1[DH:P, :], OP.mult)
        ps_d1.close()
        ps_d0.close()
        ps_cb.close()
        ps_ca.close()
        ps_s.close()
        ph2.close()
        p_qkv.close()

        # ---- Phase 3: attention dense + residual + LN1 + transpose, per token
        # chunk (keeps PE fed: next chunk's matmuls overlap LN1/transposes) ----
        p_atok = _Pool(tc, "atok", 1)
        a_tok = p_atok.tile([P, TCH, D], F32, tag="a_tok")
        p_aT = _Pool(tc, "aT", 1)
        aT = p_aT.tile([P, DSUB, T], F32R, tag="aT")
        ph3 = _Pool(tc, "ph3", 2)
        ph3x = _Pool(tc, "ph3x", 3)
        ps3 = _Pool(tc, "ps3", 2, space="PSUM")
        ps4 = _Pool(tc, "ps4", 2, space="PSUM")
        ln1 = _Pool(tc, "ln1", 4)
        wor = wo_d.ap().rearrange("(ks p) m -> p ks m", p=P)
        wo_t = []
        for jh in range(NTH):
            wt = ph3.tile([P, DSUB, NT], F32R, tag="w_o")
            nc.gpsimd.dma_start(wt[:], wor[:, :, jh * NT:(jh + 1) * NT])
            wo_t.append(wt)
        for tc_i in range(TCH):
            xres = ph3x.tile([P, D], F32, tag="xres")
            nc.sync.dma_start(xres[:], x_d.ap()[tc_i * P:(tc_i + 1) * P, :])
            psa = ps3.tile([P, NT], F32, tag="ps_oA")
            psb = ps3.tile([P, NT], F32, tag="ps_oB")
            for ks in range(DSUB):
                lhs = ctxT[:, ks, tc_i * P:(tc_i + 1) * P]
                nc.tensor.matmul(psa[:], lhs, wo_t[0][:, ks, :],
                                 start=(ks == 0), stop=(ks == DSUB - 1))
                nc.tensor.matmul(psb[:], lhs, wo_t[1][:, ks, :],
                                 start=(ks == 0), stop=(ks == DSUB - 1))
            row = a_tok[:, tc_i, :]
            nc.vector.tensor_tensor(row[:, 0:NT], psa[:], bo_b[:, 0:NT], OP.add)
            nc.vector.tensor_tensor(row[:, NT:D], psb[:], bo_b[:, NT:D], OP.add)
            nc.vector.tensor_tensor(row, row, xres[:], OP.add)
            _layer_norm_rows(nc, ln1, row, g1_b, b1_b, eps_col, "ln1")
            for ds in range(DSUB):
                pst = ps4.tile([P, P], F32, tag="pst4")
                nc.tensor.transpose(pst[:], row[:, ds * P:(ds + 1) * P], ident[:])
                nc.vector.tensor_copy(aT[:, ds, tc_i * P:(tc_i + 1) * P], pst[:])
        ln1.close()
        ps4.close()
        ps3.close()
        ph3x.close()
        ph3.close()

        # a_tok now becomes the output accumulator: out = a_tok + bo2 + FFN
        for tc_i in range(TCH):
            row = a_tok[:, tc_i, :]
            nc.vector.tensor_tensor(row, row, bo2_b[:], OP.add)

        # ---- Phase 4: FFN, f dimension processed in quarters; the final
        # quarter fuses LN2 + output store per token chunk ----
        NFR = 4
        FSH = F // NFR // P            # 8 subtiles per round
        p_int = _Pool(tc, "inter", 1)
        interT = p_int.tile([P, FSH, T], F32R, tag="interT")
        ph5 = _Pool(tc, "ph5", 2)
        ph5w = _Pool(tc, "ph5w", 3)
        ps5i = _Pool(tc, "ps5i", 2, space="PSUM")
        ps5o = _Pool(tc, "ps5o", 2, space="PSUM")
        wo2r = wo2_d.ap().rearrange("(ks p) m -> p ks m", p=P)
        wir = wi_d.ap().rearrange("(ks p) m -> p ks m", p=P)
        for fh in range(NFR):
            # intermediate: interT = gelu(aT.T @ Wi + bi)^T   (feature-major)
            for fs in range(FSH):
                fchunk = fh * FSH + fs
                wt = ph5w.tile([P, DSUB, P], F32R, tag="w_i")
                nc.gpsimd.dma_start(wt[:], wir[:, :, fchunk * P:(fchunk + 1) * P])
                psa = ps5i.tile([P, NT], F32, tag="ps_iA")
                psb = ps5i.tile([P, NT], F32, tag="ps_iB")
                for ks in range(DSUB):
                    nc.tensor.matmul(psa[:], wt[:, ks, :], aT[:, ks, 0:NT],
                                     start=(ks == 0), stop=(ks == DSUB - 1))
                    nc.tensor.matmul(psb[:], wt[:, ks, :], aT[:, ks, NT:T],
                                     start=(ks == 0), stop=(ks == DSUB - 1))
                nc.scalar.activation(interT[:, fs, 0:NT], psa[:], AF.Gelu,
                                     bias=bi_col[:, fchunk:fchunk + 1], scale=1.0)
                nc.scalar.activation(interT[:, fs, NT:T], psb[:], AF.Gelu,
                                     bias=bi_col[:, fchunk:fchunk + 1], scale=1.0)
            # output: accumulate interT.T @ Wo2 into a_tok (token-major)
            w2_t = []
            for jh in range(NTH):
                wt2 = ph5.tile([P, FSH, NT], F32R, tag="w_o2")
                nc.gpsimd.dma_start(
                    wt2[:], wo2r[:, fh * FSH:(fh + 1) * FSH, jh * NT:(jh + 1) * NT])
                w2_t.append(wt2)
            for tc_i in range(TCH):
                psa = ps5o.tile([P, NT], F32, tag="ps_o2A")
                psb = ps5o.tile([P, NT], F32, tag="ps_o2B")
                for ks in range(FSH):
                    lhs = interT[:, ks, tc_i * P:(tc_i + 1) * P]
                    nc.tensor.matmul(psa[:], lhs, w2_t[0][:, ks, :],
                                     start=(ks == 0), stop=(ks == FSH - 1))
                    nc.tensor.matmul(psb[:], lhs, w2_t[1][:, ks, :],
                                     start=(ks == 0), stop=(ks == FSH - 1))
                row = a_tok[:, tc_i, :]
                nc.vector.tensor_tensor(row[:, 0:NT], row[:, 0:NT], psa[:], OP.add)
                nc.vector.tensor_tensor(row[:, NT:D], row[:, NT:D], psb[:], OP.add)
                if fh == NFR - 1:
                    _layer_norm_rows(nc, ph5, row, g2_b, b2_b, eps_col, "ln2")
                    nc.sync.dma_start(y_d.ap()[tc_i * P:(tc_i + 1) * P, :], row)
        ps5o.close()
        ps5i.close()
        ph5w.close()
        ph5.close()
        p_int.close()
        p_aT.close()
        p_atok.close()
        p_fm.close()
    const.close()


def build_nc(loop_n=None):
    nc = bacc.Bacc("TRN2", num_devices=NCORES)
    with tile.TileContext(nc) as tc:
        build_bert_layer(tc, loop_n=loop_n)
    nc.compile()
    return nc


_CACHE = {}


def make_in_maps(hidden_states, attention_mask, Wq, bq, Wk, bk, Wv, bv, Wo, bo,
                 ln1_g, ln1_b, Wi, bi, Wo2, bo2, ln2_g, ln2_b):
    common = {
        "Wq": np.asarray(Wq, np.float32), "bq": np.asarray(bq, np.float32),
        "Wk": np.asarray(Wk, np.float32), "bk": np.asarray(bk, np.float32),
        "Wv": np.asarray(Wv, np.float32), "bv": np.asarray(bv, np.float32),
        "Wo": np.asarray(Wo, np.float32), "bo": np.asarray(bo, np.float32),
        "ln1_g": np.asarray(ln1_g, np.float32), "ln1_b": np.asarray(ln1_b, np.float32),
        "Wi": np.asarray(Wi, np.float32), "bi": np.asarray(bi, np.float32),
        "Wo2": np.asarray(Wo2, np.float32), "bo2": np.asarray(bo2, np.float32),
        "ln2_g": np.asarray(ln2_g, np.float32), "ln2_b": np.asarray(ln2_b, np.float32),
    }
    x = np.asarray(hidden_states, np.float32).reshape(B, S, D)
    m = np.asarray(attention_mask, np.float32).reshape(B, S)
    in_maps = []
    for c in range(NCORES):
        in_maps.append({
            "x": np.ascontiguousarray(x[c * BPC:(c + 1) * BPC].reshape(T, D)),
            "mask": np.ascontiguousarray(m[c * BPC:(c + 1) * BPC]),
            **common,
        })
    return in_maps


def kernel(**inputs) -> np.ndarray:
    if "nc" not in _CACHE:
        _CACHE["nc"] = build_nc()
    nc = _CACHE["nc"]
    in_maps = make_in_maps(**inputs)
    res = run_bass_kernel_spmd(nc, in_maps, core_ids=list(range(NCORES)))
    out = np.concatenate([res.results[c]["y"] for c in range(NCORES)], axis=0)
    return out.reshape(B, S, D)


# revision 30
# speedup vs baseline: 1.3172x; 1.0201x over previous
"""BERT encoder layer on 8 TRN2 NeuronCores (Bass/Tile), data-parallel over batch.

Full inputs: hidden_states [16, 512, 1024], attention_mask [16, 512], weights.
Each core processes 2 batch items (1024 tokens). Weights are replicated; no
collectives. All matmuls run in float32r (~1e-4 rel err vs the fp32 reference).

Layout strategy: activations flow feature-major ("T" suffix = [feature, token])
through the attention block, so stored [in,out] weight matrices are directly
the matmul stationary operand (out = lhsT.T @ rhs) and no weight transposes are
needed. Scores are computed transposed [kt, qt] so the attention mask is a
per-partition bias on the Exp activation, and softmax normalization is deferred:
v is augmented with a ones column so the ctx matmul simultaneously produces the
softmax denominator. LayerNorms run token-major (bn_stats over the free dim);
the two layout flips (input x, and attn_out before the FFN) use PE transposes.

Perf-critical detail (measured on HW): an fp32r matmul whose stationary operand
(lhsT) changes every instruction runs at ~2-5x the ideal 512-cycle spacing,
while back-to-back matmuls REUSING the same lhsT run at ~1.05x ideal. All dense
projections therefore interleave two PSUM accumulation groups so each lhsT tile
is consumed by two consecutive matmuls (the two 512-token/feature halves).
"""

import contextlib

import numpy as np

import concourse.bass as bass
import concourse.mybir as mybir
import concourse.tile as tile
from concourse import bacc
from concourse.bass_utils import run_bass_kernel_spmd
from concourse.masks import make_identity

F32 = mybir.dt.float32
F32R = mybir.dt.float32r
AF = mybir.ActivationFunctionType
OP = mybir.AluOpType

B, S, D, H, F = 16, 512, 1024, 16, 4096
DH = D // H                      # 64
LN_EPS = 1e-12
NCORES = 8
BPC = B // NCORES                # 2 batch items per core
T = BPC * S                      # 1024 tokens per core
P = 128
DSUB = D // P                    # 8
TCH = T // P                     # 8 token chunks
SCH = S // P                     # 4 key chunks per batch item
NT = 512                         # matmul moving-dim tile
NTH = T // NT                    # 2 token halves


class _Pool:
    """Manually-scoped tile pool (pools must close in LIFO stack order)."""

    def __init__(self, tc, name, bufs, space="SBUF"):
        self._cm = tc.tile_pool(name=name, bufs=bufs, space=space)
        self.pool = self._cm.__enter__()

    def tile(self, *a, **k):
        if "name" not in k:
            k["name"] = k.get("tag", "t")
        return self.pool.tile(*a, **k)

    def close(self):
        self._cm.__exit__(None, None, None)


def _load_bias_cols(nc, pool, dram_vec, n_sub, tag, scale=None):
    """[n_sub*P] DRAM vector -> [P, n_sub] SBUF (feature d -> [d%P, d//P])."""
    col = pool.tile([P, n_sub], F32, tag=tag)
    nc.sync.dma_start(col[:], dram_vec.rearrange("(c p) -> p c", p=P))
    if scale is not None:
        nc.vector.tensor_scalar_mul(col[:], col[:], scale)
    return col


def _load_bcast(nc, pool, dram_vec, tag):
    """[D] DRAM vector -> [P, D] SBUF, replicated across partitions."""
    t = pool.tile([P, dram_vec.shape[0]], F32, tag=tag)
    src = bass.AP(tensor=dram_vec.tensor, offset=dram_vec.offset,
                  ap=[[0, P]] + list(dram_vec.ap))
    nc.sync.dma_start(out=t[:], in_=src)
    return t


def _layer_norm_rows(nc, pool, row, g_b, b_b, eps_col, tag):
    """In-place LayerNorm of [P, D] token rows over the free (feature) dim."""
    st = pool.tile([P, 2, 6], F32, tag=f"{tag}_st")
    nc.vector.bn_stats(st[:, 0, :], row[:, 0:512])
    nc.vector.bn_stats(st[:, 1, :], row[:, 512:1024])
    mv = pool.tile([P, 2], F32, tag=f"{tag}_mv")
    nc.vector.bn_aggr(mv[:], st[:])
    std = pool.tile([P, 1], F32, tag=f"{tag}_std")
    nc.scalar.activation(std[:], mv[:, 1:2], AF.Sqrt, bias=eps_col[:], scale=1.0)
    nc.vector.reciprocal(std[:], std[:])
    nc.vector.tensor_scalar(row, row, mv[:, 0:1], std[:], OP.subtract, OP.mult)
    nc.vector.tensor_tensor(row, row, g_b[:], OP.mult)
    nc.vector.tensor_tensor(row, row, b_b[:], OP.add)


def build_bert_layer(tc, loop_n=None):
    nc = tc.nc
    dt = nc.dram_tensor
    x_d = dt("x", [T, D], F32, kind="ExternalInput")
    mask_d = dt("mask", [BPC, S], F32, kind="ExternalInput")
    wq_d = dt("Wq", [D, D], F32, kind="ExternalInput")
    bq_d = dt("bq", [D], F32, kind="ExternalInput")
    wk_d = dt("Wk", [D, D], F32, kind="ExternalInput")
    bk_d = dt("bk", [D], F32, kind="ExternalInput")
    wv_d = dt("Wv", [D, D], F32, kind="ExternalInput")
    bv_d = dt("bv", [D], F32, kind="ExternalInput")
    wo_d = dt("Wo", [D, D], F32, kind="ExternalInput")
    bo_d = dt("bo", [D], F32, kind="ExternalInput")
    g1_d = dt("ln1_g", [D], F32, kind="ExternalInput")
    b1_d = dt("ln1_b", [D], F32, kind="ExternalInput")
    wi_d = dt("Wi", [D, F], F32, kind="ExternalInput")
    bi_d = dt("bi", [F], F32, kind="ExternalInput")
    wo2_d = dt("Wo2", [F, D], F32, kind="ExternalInput")
    bo2_d = dt("bo2", [D], F32, kind="ExternalInput")
    g2_d = dt("ln2_g", [D], F32, kind="ExternalInput")
    b2_d = dt("ln2_b", [D], F32, kind="ExternalInput")
    y_d = dt("y", [T, D], F32, kind="ExternalOutput")

    const = _Pool(tc, "const", 1)
    ident = const.tile([P, P], F32, tag="ident")
    make_identity(nc, ident)
    ident_r = const.tile([P, P], F32R, tag="ident_r")
    nc.vector.tensor_copy(ident_r[:], ident[:])
    zeros_f = const.tile([P, S], F32, tag="zeros_f")
    nc.vector.memset(zeros_f[:], 0.0)
    zeros_r = const.tile([P, S], F32R, tag="zeros_r")
    nc.vector.tensor_copy(zeros_r[:], zeros_f[:])
    ones_col = const.tile([P, 1], F32, tag="ones_col")
    nc.vector.memset(ones_col[:], 1.0)
    ones_r = const.tile([P, 1], F32R, tag="ones_r")
    nc.vector.tensor_copy(ones_r[:], ones_col[:])
    eps_col = const.tile([P, 1], F32, tag="eps")
    nc.vector.memset(eps_col, LN_EPS)
    # per-feature bias columns for feature-major stages (bias = per-partition)
    bqs_col = _load_bias_cols(nc, const, bq_d.ap(), DSUB, "bqs", scale=1.0 / np.sqrt(DH))
    bk_col = _load_bias_cols(nc, const, bk_d.ap(), DSUB, "bk")
    bi_col = _load_bias_cols(nc, const, bi_d.ap(), F // P, "bi")
    # per-feature vectors broadcast across partitions for token-major stages
    bv_b = _load_bcast(nc, const, bv_d.ap(), "bv_b")
    bo_b = _load_bcast(nc, const, bo_d.ap(), "bo_b")
    g1_b = _load_bcast(nc, const, g1_d.ap(), "g1_b")
    b1_b = _load_bcast(nc, const, b1_d.ap(), "b1_b")
    bo2_b = _load_bcast(nc, const, bo2_d.ap(), "bo2_b")
    g2_b = _load_bcast(nc, const, g2_d.ap(), "g2_b")
    b2_b = _load_bcast(nc, const, b2_d.ap(), "b2_b")
    # mask[b, kt] -> [kt%P, b, kt//P] so exp can take it as per-partition bias
    mask_sb = const.tile([P, BPC, SCH], F32, tag="mask")
    for b in range(BPC):
        nc.sync.dma_start(mask_sb[:, b, :],
                          mask_d.ap()[b].rearrange("(c p) -> p c", p=P))

    loop_cm = tc.For_i(0, loop_n, 1) if loop_n else contextlib.nullcontext()
    with loop_cm:
        # fm pool: one [P, DSUB, T] f32r slot shared by Xt (ph0-1), ctxT (ph2-3)
        p_fm = _Pool(tc, "fm", 1)

        # ---- Phase 0: load x token-major, PE-transpose to feature-major Xt ----
        xt = p_fm.tile([P, DSUB, T], F32R, tag="fm")  # Xt[p, ds, t] = x[t, ds*P+p]
        ph0 = _Pool(tc, "ph0", 3)
        ps0 = _Pool(tc, "ps0", 2, space="PSUM")
        for tc_i in range(TCH):
            xtok = ph0.tile([P, D], F32R, tag="xtok")
            nc.gpsimd.dma_start(xtok[:], x_d.ap()[tc_i * P:(tc_i + 1) * P, :])
            for ds in range(DSUB):
                pst = ps0.tile([P, P], F32R, tag="pst")
                nc.tensor.transpose(pst[:], xtok[:, ds * P:(ds + 1) * P], ident_r[:])
                nc.vector.tensor_copy(xt[:, ds, tc_i * P:(tc_i + 1) * P], pst[:])
        ps0.close()
        ph0.close()

        # ---- Phase 1: QKV projections (each lhsT reused by 2 matmuls) ----
        p_qkv = _Pool(tc, "qkv", 1)
        qT = p_qkv.tile([P, DSUB, T], F32R, tag="qT")
        kT = p_qkv.tile([P, DSUB, T], F32R, tag="kT")
        v_sb = p_qkv.tile([P, TCH, D], F32R, tag="v_sb")
        ph1 = _Pool(tc, "ph1", 2)
        ph1w = _Pool(tc, "ph1w", 3)
        ps1 = _Pool(tc, "ps1", 2, space="PSUM")

        for name, w_dram, dst, bias_col, scale in (
            ("q", wq_d, qT, bqs_col, 1.0 / np.sqrt(DH)),
            ("k", wk_d, kT, bk_col, 1.0),
        ):
            wr = w_dram.ap().rearrange("(ks p) m -> p ks m", p=P)
            for mo in range(DSUB):  # output-feature chunk
                wt = ph1w.tile([P, DSUB, P], F32R, tag="w_qkv")
                nc.gpsimd.dma_start(wt[:], wr[:, :, mo * P:(mo + 1) * P])
                psa = ps1.tile([P, NT], F32, tag="ps_qA")
                psb = ps1.tile([P, NT], F32, tag="ps_qB")
                for ks in range(DSUB):
                    nc.tensor.matmul(psa[:], wt[:, ks, :], xt[:, ks, 0:NT],
                                     start=(ks == 0), stop=(ks == DSUB - 1))
                    nc.tensor.matmul(psb[:], wt[:, ks, :], xt[:, ks, NT:T],
                                     start=(ks == 0), stop=(ks == DSUB - 1))
                # dst = scale*psum + scale*bias  (scale folded into bias col)
                nc.scalar.activation(dst[:, mo, 0:NT], psa[:], AF.Identity,
                                     bias=bias_col[:, mo:mo + 1], scale=scale)
                nc.scalar.activation(dst[:, mo, NT:T], psb[:], AF.Identity,
                                     bias=bias_col[:, mo:mo + 1], scale=scale)

        # v token-major: lhsT = Xt token chunk (reused for both Wv halves)
        wvr = wv_d.ap().rearrange("(ks p) m -> p ks m", p=P)
        wv_t = []
        for jh in range(NTH):
            wvt = ph1.tile([P, DSUB, NT], F32R, tag="w_v")
            nc.gpsimd.dma_start(wvt[:], wvr[:, :, jh * NT:(jh + 1) * NT])
            wv_t.append(wvt)
        for tc_i in range(TCH):
            psa = ps1.tile([P, NT], F32, tag="ps_vA")
            psb = ps1.tile([P, NT], F32, tag="ps_vB")
            for ks in range(DSUB):
                lhs = xt[:, ks, tc_i * P:(tc_i + 1) * P]
                nc.tensor.matmul(psa[:], lhs, wv_t[0][:, ks, :],
                                 start=(ks == 0), stop=(ks == DSUB - 1))
                nc.tensor.matmul(psb[:], lhs, wv_t[1][:, ks, :],
                                 start=(ks == 0), stop=(ks == DSUB - 1))
            for jh, ps in ((0, psa), (1, psb)):
                nc.vector.tensor_tensor(
                    v_sb[:, tc_i, jh * NT:(jh + 1) * NT], ps[:],
                    bv_b[:, jh * NT:(jh + 1) * NT], OP.add)
        ps1.close()
        ph1w.close()
        ph1.close()

        # ---- Phase 2: attention ----
        # Scores use K=128: lhsT is a full kT subtile (two heads stacked) and
        # the rhs is q zero-padded outside its head rows, so each kT chunk is a
        # stationary operand REUSED by both heads of the pair (fp32r matmuls
        # with a fresh lhsT each are ~5x slower than reusing ones). The ctx
        # matmuls reuse a [v_even | v_odd] pair block the same way; garbage
        # rows are simply not read back. Softmax denominators come from cheap
        # 2-column one-hot matmuls accumulated per pair.
        ctxT = p_fm.tile([P, DSUB, T], F32R, tag="fm")  # reuses the Xt slot
        ph2 = _Pool(tc, "ph2", 2)
        ps_s = _Pool(tc, "ps_s", 2, space="PSUM")
        ps_ca = _Pool(tc, "ps_ca", 2, space="PSUM")
        ps_cb = _Pool(tc, "ps_cb", 2, space="PSUM")
        ps_d0 = _Pool(tc, "ps_d0", 1, space="PSUM")
        ps_d1 = _Pool(tc, "ps_d1", 1, space="PSUM")
        for b in range(BPC):
            bs = b * S
            for hs in range(DSUB):
                # zero-padded q tiles for the even/odd head of this pair
                pad0 = ph2.tile([P, S], F32R, tag="pad0", bufs=2)
                nc.scalar.copy(pad0[DH:P, :], zeros_r[DH:P, :])
                nc.vector.tensor_copy(pad0[0:DH, :], qT[0:DH, hs, bs:bs + S])
                pad1 = ph2.tile([P, S], F32R, tag="pad1", bufs=2)
                nc.scalar.copy(pad1[0:DH, :], zeros_r[0:DH, :])
                nc.vector.tensor_copy(pad1[DH:P, :], qT[DH:P, hs, bs:bs + S])
                exps = {}
                for c in range(SCH):
                    lhs = kT[:, hs, bs + c * P:bs + (c + 1) * P]
                    for par, pad in ((0, pad0), (1, pad1)):
                        t = ps_s.tile([P, S], F32, tag="pss")
                        nc.tensor.matmul(t[:], lhs, pad[:], start=True, stop=True)
                        e = ph2.tile([P, S], F32R, tag="expT", bufs=7)
                        nc.scalar.activation(e[:], t[:], AF.Exp,
                                             bias=mask_sb[:, b, c:c + 1], scale=1.0)
                        exps[par, c] = e
                psa = ps_ca.tile([P, S], F32, tag="ps_cA")
                psb = ps_cb.tile([P, S], F32, tag="ps_cB")
                for c in range(SCH):
                    vp = v_sb[:, b * SCH + c, hs * P:(hs + 1) * P]
                    nc.tensor.matmul(psa[:], vp, exps[0, c][:],
                                     start=(c == 0), stop=(c == SCH - 1))
                    nc.tensor.matmul(psb[:], vp, exps[1, c][:],
                                     start=(c == 0), stop=(c == SCH - 1))
                psd0 = ps_d0.tile([1, S], F32, tag="ps_den0")
                psd1 = ps_d1.tile([1, S], F32, tag="ps_den1")
                for c in range(SCH):
                    nc.tensor.matmul(psd0[:], ones_r[:], exps[0, c][:],
                                     start=(c == 0), stop=(c == SCH - 1))
                    nc.tensor.matmul(psd1[:], ones_r[:], exps[1, c][:],
                                     start=(c == 0), stop=(c == SCH - 1))
                den0 = ph2.tile([1, S], F32, tag="den0")
                nc.scalar.copy(den0[:], psd0[:])
                den1 = ph2.tile([1, S], F32, tag="den1")
                nc.scalar.copy(den1[:], psd1[:])
                rec0 = ph2.tile([DH, S], F32, tag="rec0", bufs=2)
                nc.gpsimd.partition_broadcast(rec0[:], den0[:])
                rec1 = ph2.tile([P, S], F32, tag="rec1", bufs=2)
                nc.gpsimd.partition_broadcast(rec1[:], den1[:])
                nc.vector.reciprocal(rec0[:, :], rec0[:, :])
                nc.vector.reciprocal(rec1[DH:P, :], rec1[DH:P, :])
                nc.vector.tensor_tensor(ctxT[0:DH, hs, bs:bs + S],
                                        psa[0:DH, :], rec0[:, :], OP.mult)
                nc.vector.tensor_tensor(ctxT[DH:P, hs, bs:bs + S],
                                        psb[DH:P, :], rec1[DH:P, :], OP.mult)
        ps_d1.close()
        ps_d0.close()
        ps_cb.close()
        ps_ca.close()
        ps_s.close()
        ph2.close()
        p_qkv.close()

        # ---- Phase 3: attention dense + residual + LN1 + transpose, per token
        # chunk (keeps PE fed: next chunk's matmuls overlap LN1/transposes) ----
        p_atok = _Pool(tc, "atok", 1)
        a_tok = p_atok.tile([P, TCH, D], F32, tag="a_tok")
        p_aT = _Pool(tc, "aT", 1)
        aT = p_aT.tile([P, DSUB, T], F32R, tag="aT")
        ph3 = _Pool(tc, "ph3", 2)
        ph3x = _Pool(tc, "ph3x", 3)
        ps3 = _Pool(tc, "ps3", 2, space="PSUM")
        ps4 = _Pool(tc, "ps4", 2, space="PSUM")
        ln1 = _Pool(tc, "ln1", 4)
        wor = wo_d.ap().rearrange("(ks p) m -> p ks m", p=P)
        wo_t = []
        for jh in range(NTH):
            wt = ph3.tile([P, DSUB, NT], F32R, tag="w_o")
            nc.gpsimd.dma_start(wt[:], wor[:, :, jh * NT:(jh + 1) * NT])
            wo_t.append(wt)
        for tc_i in range(TCH):
            xres = ph3x.tile([P, D], F32, tag="xres")
            nc.sync.dma_start(xres[:], x_d.ap()[tc_i * P:(tc_i + 1) * P, :])
            psa = ps3.tile([P, NT], F32, tag="ps_oA")
            psb = ps3.tile([P, NT], F32, tag="ps_oB")
            for ks in range(DSUB):
                lhs = ctxT[:, ks, tc_i * P:(tc_i + 1) * P]
                nc.tensor.matmul(psa[:], lhs, wo_t[0][:, ks, :],
                                 start=(ks == 0), stop=(ks == DSUB - 1))
                nc.tensor.matmul(psb[:], lhs, wo_t[1][:, ks, :],
                                 start=(ks == 0), stop=(ks == DSUB - 1))
            row = a_tok[:, tc_i, :]
            nc.vector.tensor_tensor(row[:, 0:NT], psa[:], bo_b[:, 0:NT], OP.add)
            nc.vector.tensor_tensor(row[:, NT:D], psb[:], bo_b[:, NT:D], OP.add)
            nc.vector.tensor_tensor(row, row, xres[:], OP.add)
            _layer_norm_rows(nc, ln1, row, g1_b, b1_b, eps_col, "ln1")
            for ds in range(DSUB):
                pst = ps4.tile([P, P], F32, tag="pst4")
                nc.tensor.transpose(pst[:], row[:, ds * P:(ds + 1) * P], ident[:])
                nc.vector.tensor_copy(aT[:, ds, tc_i * P:(tc_i + 1) * P], pst[:])
        ln1.close()
        ps4.close()
        ps3.close()
        ph3x.close()
        ph3.close()

        # a_tok now becomes the output accumulator: out = a_tok + bo2 + FFN
        for tc_i in range(TCH):
            row = a_tok[:, tc_i, :]
            nc.vector.tensor_tensor(row, row, bo2_b[:], OP.add)

        # ---- Phase 4: FFN, f dimension processed in quarters; the final
        # quarter fuses LN2 + output store per token chunk ----
        NFR = 4
        FSH = F // NFR // P            # 8 subtiles per round
        p_int = _Pool(tc, "inter", 1)
        interT = p_int.tile([P, FSH, T], F32R, tag="interT")
        ph5 = _Pool(tc, "ph5", 2)
        ph5w = _Pool(tc, "ph5w", 3)
        ps5i = _Pool(tc, "ps5i", 2, space="PSUM")
        ps5o = _Pool(tc, "ps5o", 2, space="PSUM")
        wo2r = wo2_d.ap().rearrange("(ks p) m -> p ks m", p=P)
        wir = wi_d.ap().rearrange("(ks p) m -> p ks m", p=P)
        for fh in range(NFR):
            # intermediate: interT = gelu(aT.T @ Wi + bi)^T   (feature-major)
            for fs in range(FSH):
                fchunk = fh * FSH + fs
                wt = ph5w.tile([P, DSUB, P], F32R, tag="w_i")
                nc.gpsimd.dma_start(wt[:], wir[:, :, fchunk * P:(fchunk + 1) * P])
                psa = ps5i.tile([P, NT], F32, tag="ps_iA")
                psb = ps5i.tile([P, NT], F32, tag="ps_iB")
                for ks in range(DSUB):
                    nc.tensor.matmul(psa[:], wt[:, ks, :], aT[:, ks, 0:NT],
                                     start=(ks == 0), stop=(ks == DSUB - 1))
                    nc.tensor.matmul(psb[:], wt[:, ks, :], aT[:, ks, NT:T],
                                     start=(ks == 0), stop=(ks == DSUB - 1))
                nc.scalar.activation(interT[:, fs, 0:NT], psa[:], AF.Gelu,
                                     bias=bi_col[:, fchunk:fchunk + 1], scale=1.0)
                nc.scalar.activation(interT[:, fs, NT:T], psb[:], AF.Gelu,
                                     bias=bi_col[:, fchunk:fchunk + 1], scale=1.0)
            # output: accumulate interT.T @ Wo2 into a_tok (token-major)
            w2_t = []
            for jh in range(NTH):
                wt2 = ph5.tile([P, FSH, NT], F32R, tag="w_o2")
                nc.gpsimd.dma_start(
                    wt2[:], wo2r[:, fh * FSH:(fh + 1) * FSH, jh * NT:(jh + 1) * NT])
                w2_t.append(wt2)
            for tc_i in range(TCH):
                psa = ps5o.tile([P, NT], F32, tag="ps_o2A")
                psb = ps5o.tile([P, NT], F32, tag="ps_o2B")
                for ks in range(FSH):
                    lhs = interT[:, ks, tc_i * P:(tc_i + 1) * P]
                    nc.tensor.matmul(psa[:], lhs, w2_t[0][:, ks, :],
                                     start=(ks == 0), stop=(ks == FSH - 1))
                    nc.tensor.matmul(psb[:], lhs, w2_t[1][:, ks, :],
                                     start=(ks == 0), stop=(ks == FSH - 1))
                row = a_tok[:, tc_i, :]
                nc.vector.tensor_tensor(row[:, 0:NT], row[:, 0:NT], psa[:], OP.add)
                nc.vector.tensor_tensor(row[:, NT:D], row[:, NT:D], psb[:], OP.add)
                if fh == NFR - 1:
                    _layer_norm_rows(nc, ph5, row, g2_b, b2_b, eps_col, "ln2")
                    nc.sync.dma_start(y_d.ap()[tc_i * P:(tc_i + 1) * P, :], row)
        ps5o.close()
        ps5i.close()
        ph5w.close()
        ph5.close()
        p_int.close()
        p_aT.close()
        p_atok.close()
        p_fm.close()
    const.close()


def build_nc(loop_n=None):
    nc = bacc.Bacc("TRN2", num_devices=NCORES)
    with tile.TileContext(nc) as tc:
        build_bert_layer(tc, loop_n=loop_n)
    nc.compile()
    return nc


_CACHE = {}


def make_in_maps(hidden_states, attention_mask, Wq, bq, Wk, bk, Wv, bv, Wo, bo,
                 ln1_g, ln1_b, Wi, bi, Wo2, bo2, ln2_g, ln2_b):
    common = {
        "Wq": np.asarray(Wq, np.float32), "bq": np.asarray(bq, np.float32),
        "Wk": np.asarray(Wk, np.float32), "bk": np.asarray(bk, np.float32),
        "Wv": np.asarray(Wv, np.float32), "bv": np.asarray(bv, np.float32),
        "Wo": np.asarray(Wo, np.float32), "bo": np.asarray(bo, np.float32),
        "ln1_g": np.asarray(ln1_g, np.float32), "ln1_b": np.asarray(ln1_b, np.float32),
        "Wi": np.asarray(Wi, np.float32), "bi": np.asarray(bi, np.float32),
        "Wo2": np.asarray(Wo2, np.float32), "bo2": np.asarray(bo2, np.float32),
        "ln2_g": np.asarray(ln2_g, np.float32), "ln2_b": np.asarray(ln2_b, np.float32),
    }
    x = np.asarray(hidden_states, np.float32).reshape(B, S, D)
    m = np.asarray(attention_mask, np.float32).reshape(B, S)
    in_maps = []
    for c in range(NCORES):
        in_maps.append({
            "x": np.ascontiguousarray(x[c * BPC:(c + 1) * BPC].reshape(T, D)),
            "mask": np.ascontiguousarray(m[c * BPC:(c + 1) * BPC]),
            **common,
        })
    return in_maps


def kernel(**inputs) -> np.ndarray:
    if "nc" not in _CACHE:
        _CACHE["nc"] = build_nc()
    nc = _CACHE["nc"]
    in_maps = make_in_maps(**inputs)
    res = run_bass_kernel_spmd(nc, in_maps, core_ids=list(range(NCORES)))
    out = np.concatenate([res.results[c]["y"] for c in range(NCORES)], axis=0)
    return out.reshape(B, S, D)
